# revision 1
# baseline (speedup 1.0000x reference)
"""Distributed FFT (N = 2^24 complex points) on 8 Trainium2 NeuronCores.

Four-step (Cooley-Tukey) decomposition N = 4096 x 4096:
  launch 1: per global column j1g, FFT_4096 over j2g      (batch parallel over j1g)
  host:     global twiddle wN^{j1g*k2g} + transpose exchange
  launch 2: per global row k2g, FFT_4096 over j1g         (batch parallel over k2g)

Both launches run the SAME compiled SPMD kernel on all 8 cores: a batch of
512 local FFT_4096 per core. Each FFT_4096 = radix-32 stage (block-diag 4x
packed over the contraction axis, K=128) fused with its inter-stage transpose
(data-stationary matmul: psum[j1, :] += S_slice.T @ [[Wr|Wi],[-Wi|Wr]]),
then a radix-128 stage whose twiddle exp(-2pi i j1 kap2/4096) is folded into
32 per-kap2 weight matrices {Br, Bi, -Bi}. All arithmetic is float32r
TensorE matmuls with fp32 PSUM accumulation; all moving dims are >= 256 for
full-rate f32r.

Local FFT_4096 digits: f = j1 + 128*j2 (j1 in [0,128) fast, j2 in [0,32));
k = kap2 + 32*kap1. Batch b = 128*t + 32*g + s (t chunk of 128, g K-pack
group, s in [0,32)). Host does all layout marshalling (numpy index shuffles);
device sees only contiguous [128, X] DMAs.
"""
import numpy as np

import concourse.mybir as mybir
import concourse.tile as tile
from concourse import bacc
from concourse.bass_utils import run_bass_kernel_spmd

NG = 4096                 # global matrix dimension; N = NG*NG
N = NG * NG
NCORES = 8
BPC = NG // NCORES        # 512 signals per core per launch
import os as _os
NCHUNK = 4                # chunks of 128 signals
_ABLATE = ""              # debug ablations disabled in the shipped kernel
USE_F32R = True           # float32r matmuls for stages A/B (4x faster than fp32)

_F32 = mybir.dt.float32
_F32R = mybir.dt.float32r

# ---------------------------------------------------------------------------
# constants (host-side numpy)
# ---------------------------------------------------------------------------

_consts_cache = None


def _make_consts():
    global _consts_cache
    if _consts_cache is not None:
        return _consts_cache
    j2 = np.arange(32)
    W32 = np.exp(-2j * np.pi * np.outer(j2, j2) / 32)
    I4 = np.eye(4)
    BDr = np.kron(I4, W32.real)
    BDi = np.kron(I4, W32.imag)
    # moving-operand matrices for the fused stageA+transpose matmuls:
    #   psum[j1, 0:128] = Fr, psum[j1, 128:256] = Fi  (accumulated over Sr, Si)
    bdc = np.stack([
        np.concatenate([BDr, BDi], axis=1),     # applied to Sr
        np.concatenate([-BDi, BDr], axis=1),    # applied to Si
    ]).astype(np.float32)                       # [2,128,256]

    j1 = np.arange(128)
    W128 = np.exp(-2j * np.pi * np.outer(j1, j1) / 128)
    bm = np.zeros((32, 3, 128, 128), np.float32)
    for kap2 in range(32):
        B = np.exp(-2j * np.pi * j1 * kap2 / 4096)[:, None] * W128  # [j1][kap1]
        bm[kap2, 0] = B.real
        bm[kap2, 1] = B.imag
        bm[kap2, 2] = -B.imag
    _consts_cache = (bdc, bm)
    return _consts_cache


_tw_cache = None


def _global_twiddle():
    """exp(-2pi i k2g*j1g / N) as complex64 [NG, NG] (k2g rows)."""
    global _tw_cache
    if _tw_cache is None:
        k = np.arange(NG, dtype=np.float64)
        phase = np.outer(k, k) * (-2.0 * np.pi / N)
        _tw_cache = np.exp(1j * phase).astype(np.complex64)
    return _tw_cache


# ---------------------------------------------------------------------------
# marshalling (host)
# ---------------------------------------------------------------------------

def _marshal_in(Vre, Vim):
    """Vre/Vim: [4096 f][512 b] f32 planes -> in2 [4,2,128,4096] f32."""
    out = np.empty((NCHUNK, 2, 128, 4096), np.float32)
    for pl, V in ((0, Vre), (1, Vim)):
        V2 = V.reshape(32, 128, 4, 4, 32)      # j2, j1, t, g, s
        out[:, pl] = V2.transpose(2, 3, 0, 4, 1).reshape(4, 128, 4096)
    return out


def _unmarshal_out(O):
    """out2 [2,16,2,128,512] f32 (dims sc,kp,pl,kap1,n2; n2=256u+128c2+4s+g)
    -> (Fre, Fim) planes [4096 k][512 b]."""
    O8 = O.reshape(2, 16, 2, 128, 2, 2, 32, 4)  # sc, kp, pl, kap1, u, c2, s, g
    # kap2 = 2*kp+u ; k = 32*kap1 + kap2 ; b = 256*sc + 128*c2 + 32*g + s
    P = np.ascontiguousarray(O8.transpose(2, 3, 1, 4, 0, 5, 7, 6)).reshape(2, 4096, 512)
    return P[0], P[1]


# ---------------------------------------------------------------------------
# device kernel (Bass/Tile), shared by both launches
# ---------------------------------------------------------------------------

_nc_cache = None


def _build_nc():
    global _nc_cache
    if _nc_cache is not None:
        return _nc_cache

    nc = bacc.Bacc(trn_type="TRN2")
    DT = _F32R if USE_F32R else _F32
    # in layout: [chunk, plane, p = 32g+j2, ff = 128s+j1]
    in_d = nc.dram_tensor("in2", [NCHUNK, 2, 128, 4096], DT, kind="ExternalInput")
    bdc_d = nc.dram_tensor("bdc", [2, 128, 256], DT, kind="ExternalInput")
    bm_d = nc.dram_tensor("bm", [32, 3, 128, 128], DT, kind="ExternalInput")
    # out layout: [superchunk, kap2pair, plane, kap1, n2], n2 = 128*c2 + 4*s + g
    out_d = nc.dram_tensor("out2", [NCHUNK // 2, 16, 2, 128, 512], _F32,
                           kind="ExternalOutput")

    with tile.TileContext(nc) as tc:
        with (
            tc.tile_pool(name="consts", bufs=1) as cpool,
            tc.tile_pool(name="inp", bufs=3) as inpool,
            tc.tile_pool(name="tp", bufs=1) as tpool,
            tc.tile_pool(name="outp", bufs=3) as outpool,
            tc.tile_pool(name="pA", bufs=4, space="PSUM") as pA,
            tc.tile_pool(name="pB", bufs=4, space="PSUM") as pB,
        ):
            # resident constants
            bdc_t = cpool.tile([128, 2, 256], DT, tag="bdc")
            nc.sync.dma_start(bdc_t[:], bdc_d.ap().rearrange("a p f -> p a f"))
            bm_t = cpool.tile([128, 32, 3, 128], DT, tag="bm")
            nc.sync.dma_start(bm_t[:], bm_d.ap().rearrange("a b p f -> p a b f"))

            ncopy = 0  # alternate DVE/ACT for PSUM evacuations

            def evac(out_ap, in_ap):
                nonlocal ncopy
                if ncopy % 2 == 0:
                    nc.vector.tensor_copy(out_ap, in_ap)
                else:
                    nc.scalar.copy(out_ap, in_ap)
                ncopy += 1

            for sc in range(NCHUNK // 2):
                # T for the superchunk: [p=j1][c2][s][plane][g][kap]
                tt = tpool.tile([128, 2, 32, 2, 4, 32], DT, tag="tt")
                ttf = tt.rearrange("p a b c d e -> p (a b c d e)")

                if "noa" in _ABLATE:
                    nc.vector.memset(ttf[:, :1024], 0.0)
                for c2 in range(2) if "noa" not in _ABLATE else []:
                    t = 2 * sc + c2
                    # ---- fused stage A + transpose: per s:
                    #   psum[j1, pl*128 + 32g+kap2] += S_sl.T @ bdc[pl-combo]
                    for h in range(2):
                        hs = slice(2048 * h, 2048 * h + 2048)
                        s0t = inpool.tile([128, 2048], DT, tag="inre")
                        s1t = inpool.tile([128, 2048], DT, tag="inim")
                        if "noin" not in _ABLATE:
                            nc.sync.dma_start(s0t[:], in_d[t, 0, :, hs])
                            nc.sync.dma_start(s1t[:], in_d[t, 1, :, hs])
                        for sp in range(8):      # s-pairs within half
                            bank = pA.tile([128, 512], _F32, tag="psA")
                            for e in range(2):
                                sl = 2 * sp + e          # s within half
                                ds = slice(128 * sl, 128 * sl + 128)
                                ys = slice(256 * e, 256 * e + 256)
                                nc.tensor.matmul(bank[:, ys], s0t[:, ds],
                                                 bdc_t[:, 0], start=True, stop=False)
                                nc.tensor.matmul(bank[:, ys], s1t[:, ds],
                                                 bdc_t[:, 1], start=False, stop=True)
                            s0 = 16 * h + 2 * sp         # s within chunk
                            off = (c2 * 32 + s0) * 256
                            evac(ttf[:, off:off + 512], bank[:])

                # ---- stage B: radix-128, per-kap2 twiddled weights, N=256
                for kp in range(16) if "nob" not in _ABLATE else []:  # kap2 pairs
                    yr = pB.tile([128, 512], _F32, tag="psB")
                    yi = pB.tile([128, 512], _F32, tag="psB")
                    for u in range(2):
                        kap2 = 2 * kp + u
                        ys = slice(256 * u, 256 * u + 256)
                        trs = tt[:, :, :, 0, :, kap2]
                        tis = tt[:, :, :, 1, :, kap2]
                        br = bm_t[:, kap2, 0]
                        bi = bm_t[:, kap2, 1]
                        bni = bm_t[:, kap2, 2]
                        nc.tensor.matmul(yr[:, ys], br, trs, start=True, stop=False)
                        nc.tensor.matmul(yi[:, ys], br, tis, start=True, stop=False)
                        nc.tensor.matmul(yr[:, ys], bni, tis, start=False, stop=True)
                        nc.tensor.matmul(yi[:, ys], bi, trs, start=False, stop=True)
                    # evac + store
                    ot = outpool.tile([128, 2, 512], _F32, tag="out")
                    evac(ot[:, 0], yr[:])
                    evac(ot[:, 1], yi[:])
                    if "noout" not in _ABLATE:
                        nc.sync.dma_start(out_d[sc, kp, 0], ot[:, 0])
                        nc.sync.dma_start(out_d[sc, kp, 1], ot[:, 1])

    nc.finalize()
    _nc_cache = nc
    return nc


# ---------------------------------------------------------------------------
# launch helper
# ---------------------------------------------------------------------------

_last_exec_ns = None


def last_exec_time_ns():
    """Sum of HW exec times (ns) of the launches in the last kernel() call,
    when KERNEL_TRACE=1 was set and NTFF profiling is available. None otherwise."""
    return _last_exec_ns


def predicted_exec_time_ns():
    """Cost-model (TimelineSim) predicted HW exec time for both launches, ns."""
    from concourse.timeline_sim import TimelineSim
    nc = _build_nc()
    return int(2 * TimelineSim(nc).simulate())


def _run_launch(cols_re, cols_im):
    """cols_re/cols_im: list of 8 planes [4096 f][512 b] f32.
    Returns list of 8 (Fre, Fim) planes [4096 k][512 b]."""
    global _last_exec_ns
    import os
    nc = _build_nc()
    bdc, bm = _make_consts()
    in_maps = []
    for c in range(NCORES):
        in_maps.append({
            "in2": _marshal_in(cols_re[c], cols_im[c]),
            "bdc": bdc, "bm": bm,
        })
    trace = bool(os.environ.get("KERNEL_TRACE"))
    try:
        res = run_bass_kernel_spmd(nc, in_maps, core_ids=list(range(NCORES)),
                                   trace=trace)
    except ModuleNotFoundError:
        # NTFF profiling hook unavailable under this axon client; run untraced.
        res = run_bass_kernel_spmd(nc, in_maps, core_ids=list(range(NCORES)))
    if trace and getattr(res, "exec_time_ns", None) is not None:
        _last_exec_ns = (_last_exec_ns or 0) + res.exec_time_ns
    return [_unmarshal_out(res.results[c]["out2"]) for c in range(NCORES)]


# ---------------------------------------------------------------------------
# public entry point
# ---------------------------------------------------------------------------

def kernel(x: np.ndarray) -> np.ndarray:
    """x: [N, 2] float32 (re, im). Returns FFT(x) as [N, 2] float32."""
    global _last_exec_ns
    _last_exec_ns = None
    x = np.asarray(x)
    Are = np.ascontiguousarray(x[:, 0].reshape(NG, NG))  # [j2g][j1g]
    Aim = np.ascontiguousarray(x[:, 1].reshape(NG, NG))

    # launch 1: FFT over rows (j2g) for each column j1g
    cols_re = [np.ascontiguousarray(Are[:, BPC * c:BPC * (c + 1)]) for c in range(NCORES)]
    cols_im = [np.ascontiguousarray(Aim[:, BPC * c:BPC * (c + 1)]) for c in range(NCORES)]
    l1 = _run_launch(cols_re, cols_im)

    # host: assemble F [k2g][j1g], twiddle, transpose-exchange
    F = np.empty((NG, NG), np.complex64)
    for c in range(NCORES):
        fre, fim = l1[c]
        F[:, BPC * c:BPC * (c + 1)] = fre + 1j * fim
    F *= _global_twiddle()

    # launch 2: FFT over j1g for each row k2g; core d gets rows [512d, 512(d+1))
    cols_re2 = []
    cols_im2 = []
    for d in range(NCORES):
        block = F[BPC * d:BPC * (d + 1), :].T      # [j1g][k2g-local]
        cols_re2.append(np.ascontiguousarray(block.real))
        cols_im2.append(np.ascontiguousarray(block.imag))
    l2 = _run_launch(cols_re2, cols_im2)

    # assemble Xmat [k1g][k2g]; out flat index k = 4096*k1g + k2g
    out = np.empty((NG, NG, 2), np.float32)
    for d in range(NCORES):
        rre, rim = l2[d]
        out[:, BPC * d:BPC * (d + 1), 0] = rre
        out[:, BPC * d:BPC * (d + 1), 1] = rim
    return out.reshape(N, 2)



# revision 27
# speedup vs baseline: 1.8243x; 1.8243x over previous
"""Distributed FFT (N = 2^24 complex points) on 8 Trainium2 NeuronCores.

Four-step (Cooley-Tukey) decomposition N = 4096 x 4096:
  launch 1: per global column j1g, FFT_4096 over j2g      (batch parallel over j1g)
  host:     global twiddle wN^{j1g*k2g} + transpose exchange
  launch 2: per global row k2g, FFT_4096 over j1g         (batch parallel over k2g)

Both launches run the SAME compiled SPMD kernel on all 8 cores: a batch of
512 local FFT_4096 per core. Each FFT_4096 = radix-32 stage (block-diag 4x
packed over the contraction axis, K=128) fused with its inter-stage transpose
(data-stationary matmul: psum[j1, :] += S_slice.T @ [[Wr|Wi],[-Wi|Wr]]),
then a radix-128 stage whose twiddle exp(-2pi i j1 kap2/4096) is folded into
32 per-kap2 weight matrices {Br, Bi, -Bi}. All arithmetic is float32r
TensorE matmuls with fp32 PSUM accumulation; all moving dims are >= 256 for
full-rate f32r.

Local FFT_4096 digits: f = j1 + 128*j2 (j1 in [0,128) fast, j2 in [0,32));
k = kap2 + 32*kap1. Batch b = 128*t + 32*g + s (t chunk of 128, g K-pack
group, s in [0,32)). Host does all layout marshalling (numpy index shuffles);
device sees only contiguous [128, X] DMAs.
"""
import numpy as np

import concourse.mybir as mybir
import concourse.tile as tile
from concourse import bacc
from concourse.bass_utils import run_bass_kernel_spmd

NG = 4096                 # global matrix dimension; N = NG*NG
N = NG * NG
NCORES = 8
BPC = NG // NCORES        # 512 signals per core per launch
import os as _os
NCHUNK = 4                # chunks of 128 signals
_ABLATE = ""              # debug ablations disabled in the shipped kernel

import ml_dtypes
_BF16_NP = ml_dtypes.bfloat16

_F32 = mybir.dt.float32
_BF16 = mybir.dt.bfloat16

# ---------------------------------------------------------------------------
# constants (host-side numpy)
# ---------------------------------------------------------------------------

_consts_cache = None


def _make_consts():
    global _consts_cache
    if _consts_cache is not None:
        return _consts_cache
    j2 = np.arange(32)
    W32 = np.exp(-2j * np.pi * np.outer(j2, j2) / 32)
    I4 = np.eye(4)
    BDr = np.kron(I4, W32.real)
    BDi = np.kron(I4, W32.imag)
    # moving-operand matrices for the fused stageA+transpose matmuls:
    #   psum[j1, 0:128] = Fr, psum[j1, 128:256] = Fi  (accumulated over Sr, Si)
    bdc = np.stack([
        np.concatenate([BDr, BDi], axis=1),     # applied to Sr
        np.concatenate([-BDi, BDr], axis=1),    # applied to Si
    ]).astype(_BF16_NP)                         # [2,128,256]

    j1 = np.arange(128)
    W128 = np.exp(-2j * np.pi * np.outer(j1, j1) / 128)
    bm = np.zeros((32, 3, 128, 128), np.float32)
    for kap2 in range(32):
        B = np.exp(-2j * np.pi * j1 * kap2 / 4096)[:, None] * W128  # [j1][kap1]
        bm[kap2, 0] = B.real
        bm[kap2, 1] = B.imag
        bm[kap2, 2] = -B.imag
    # p-major DRAM layout [p=128, kap2=32, v=3, kap1=128] so the device DMA
    # is one contiguous run per partition (24 KiB in bf16).
    bm = np.ascontiguousarray(bm.transpose(2, 0, 1, 3)).astype(_BF16_NP)
    _consts_cache = (bdc, bm)
    return _consts_cache


_tw_cache = None


def _global_twiddle():
    """exp(-2pi i k2g*j1g / N) as complex64 [NG, NG] (k2g rows)."""
    global _tw_cache
    if _tw_cache is None:
        k = np.arange(NG, dtype=np.float64)
        phase = np.outer(k, k) * (-2.0 * np.pi / N)
        _tw_cache = np.exp(1j * phase).astype(np.complex64)
    return _tw_cache


# ---------------------------------------------------------------------------
# marshalling (host)
# ---------------------------------------------------------------------------

def _marshal_in(Vre, Vim):
    """Vre/Vim: [4096 f][512 b] f32 planes -> in2 [4,2,128,4096] bf16."""
    out = np.empty((NCHUNK, 2, 128, 4096), _BF16_NP)
    for pl, V in ((0, Vre), (1, Vim)):
        V2 = V.reshape(32, 128, 4, 4, 32)      # j2, j1, t, g, s
        out[:, pl] = V2.transpose(2, 3, 0, 4, 1).reshape(4, 128, 4096).astype(_BF16_NP)
    return out


def _unmarshal_out(O):
    """out2 [2,16,2,128,512] bf16 (dims sc,kp,pl,kap1,n2; n2=256u+128c2+4s+g)
    -> (Fre, Fim) planes [4096 k][512 b]."""
    O = np.asarray(O).astype(np.float32)
    O8 = O.reshape(2, 16, 2, 128, 2, 2, 32, 4)  # sc, kp, pl, kap1, u, c2, s, g
    # kap2 = 2*kp+u ; k = 32*kap1 + kap2 ; b = 256*sc + 128*c2 + 32*g + s
    P = np.ascontiguousarray(O8.transpose(2, 3, 1, 4, 0, 5, 7, 6)).reshape(2, 4096, 512)
    return P[0], P[1]


# ---------------------------------------------------------------------------
# device kernel (Bass/Tile), shared by both launches
# ---------------------------------------------------------------------------

_nc_cache = None


def _build_nc():
    global _nc_cache
    if _nc_cache is not None:
        return _nc_cache

    nc = bacc.Bacc(trn_type="TRN2")
    DT = _BF16
    # in layout: [chunk, plane, p = 32g+j2, ff = 128s+j1]
    in_d = nc.dram_tensor("in2", [NCHUNK, 2, 128, 4096], DT, kind="ExternalInput")
    bdc_d = nc.dram_tensor("bdc", [2, 128, 256], DT, kind="ExternalInput")
    bm_d = nc.dram_tensor("bm", [128, 32, 3, 128], DT, kind="ExternalInput")
    # out layout: [superchunk, kap2pair, plane, kap1, n2], n2 = 128*c2 + 4*s + g
    out_d = nc.dram_tensor("out2", [NCHUNK // 2, 16, 2, 128, 512], DT,
                           kind="ExternalOutput")

    with tile.TileContext(nc) as tc:
        with (
            tc.tile_pool(name="consts", bufs=1) as cpool,
            tc.tile_pool(name="inp", bufs=3) as inpool,
            tc.tile_pool(name="tp", bufs=2) as tpool,
            tc.tile_pool(name="outp", bufs=8) as outpool,
            tc.tile_pool(name="pA", bufs=4, space="PSUM") as pA,
            tc.tile_pool(name="pB", bufs=4, space="PSUM") as pB,
        ):
            st = {}

            def load_chunk(t, first=False):
                if t in st or t >= NCHUNK:
                    return
                s = inpool.tile([128, 2, 4096], DT, tag="in")
                # quarter-DMAs so stage A can start on the first piece early;
                # for chunk 0 an even smaller first eighth, with the (tiny)
                # bdc const DMA slotted right after it.
                if first:
                    for q in range(2):
                        qs = slice(512 * q, 512 * q + 512)
                        nc.sync.dma_start(s[:, :, qs],
                                          in_d[t, :, :, qs].rearrange("a p f -> p a f"))
                        if q == 0:
                            nc.sync.dma_start(
                                bdc_t[:], bdc_d.ap().rearrange("a p f -> p a f"))
                    qlist = range(1, 4)
                else:
                    qlist = range(4)
                for q in qlist:
                    qs = slice(1024 * q, 1024 * q + 1024)
                    nc.sync.dma_start(s[:, :, qs],
                                      in_d[t, :, :, qs].rearrange("a p f -> p a f"))
                st[t] = s

            bdc_t = cpool.tile([128, 2, 256], DT, tag="bdc")
            load_chunk(0, first=True)
            load_chunk(1)

            # bm streams in 4 kap2-groups interleaved with chunk prefetches,
            # each arriving just before stage B consumes it.
            bm_t = cpool.tile([128, 32, 3, 128], DT, tag="bm")
            bm_loaded = [False] * 4

            def load_bm_group(g):
                if g >= 4 or bm_loaded[g]:
                    return
                ks = slice(8 * g, 8 * g + 8)
                nc.sync.dma_start(bm_t[:, ks], bm_d[:, ks])
                bm_loaded[g] = True

            load_bm_group(0)
            load_bm_group(1)
            load_bm_group(2)

            ncopyA = 0  # stage A: rotate DVE/ACT/Pool
            ncopyB = 0  # stage B: rotate DVE/ACT (latency-critical)

            def evacA(out_ap, in_ap):
                nonlocal ncopyA
                if ncopyA % 2 == 0:
                    nc.vector.tensor_copy(out_ap, in_ap)
                else:
                    nc.scalar.copy(out_ap, in_ap)
                ncopyA += 1

            def evacB(out_ap, in_ap):
                nonlocal ncopyB
                if ncopyB % 2 == 0:
                    nc.vector.tensor_copy(out_ap, in_ap)
                else:
                    nc.scalar.copy(out_ap, in_ap)
                ncopyB += 1

            for sc in range(NCHUNK // 2):
                # T for the superchunk: [p=j1][c2][s][plane][g][kap]
                tt = tpool.tile([128, 2, 32, 2, 4, 32], DT, tag="tt")
                ttf = tt.rearrange("p a b c d e -> p (a b c d e)")

                for c2 in range(2):
                    t = 2 * sc + c2
                    load_chunk(t)
                    load_chunk(t + 1)   # prefetch
                    s = st.pop(t)
                    # ---- fused stage A + transpose: per s:
                    #   psum[j1, pl*128 + 32g+kap2] += S_sl.T @ bdc[pl-combo]
                    for sp in range(16):     # s-pairs within chunk
                        bank = pA.tile([128, 512], _F32, tag="psA")
                        for e in range(2):
                            sl = 2 * sp + e          # s within chunk
                            ds = slice(128 * sl, 128 * sl + 128)
                            ys = slice(256 * e, 256 * e + 256)
                            nc.tensor.matmul(bank[:, ys], s[:, 0, ds],
                                             bdc_t[:, 0], start=True, stop=False)
                            nc.tensor.matmul(bank[:, ys], s[:, 1, ds],
                                             bdc_t[:, 1], start=False, stop=True)
                        off = (c2 * 32 + 2 * sp) * 256
                        evacA(ttf[:, off:off + 512], bank[:])

                # ---- stage B: radix-128, per-kap2 twiddled weights, N=256
                if sc == 0:
                    load_bm_group(3)
                    load_chunk(3)       # ahead of this sc's output DMAs
                for kp in range(16):     # kap2 pairs
                    ot = outpool.tile([128, 2, 512], DT, tag="out")
                    yr = pB.tile([128, 512], _F32, tag="psB")
                    yi = pB.tile([128, 512], _F32, tag="psB")
                    for u in range(2):
                        kap2 = 2 * kp + u
                        ys = slice(256 * u, 256 * u + 256)
                        trs = tt[:, :, :, 0, :, kap2]
                        tis = tt[:, :, :, 1, :, kap2]
                        br = bm_t[:, kap2, 0]
                        bi = bm_t[:, kap2, 1]
                        bni = bm_t[:, kap2, 2]
                        nc.tensor.matmul(yr[:, ys], br, trs, start=True, stop=False)
                        nc.tensor.matmul(yi[:, ys], br, tis, start=True, stop=False)
                        nc.tensor.matmul(yr[:, ys], bni, tis, start=False, stop=True)
                        nc.tensor.matmul(yi[:, ys], bi, trs, start=False, stop=True)
                    evacB(ot[:, 0], yr[:])
                    evacB(ot[:, 1], yi[:])
                    nc.sync.dma_start(
                        out_d[sc, kp].rearrange("a p f -> p a f"), ot[:])

    nc.finalize()
    _nc_cache = nc
    return nc


# ---------------------------------------------------------------------------
# launch helper
# ---------------------------------------------------------------------------

_last_exec_ns = None


def last_exec_time_ns():
    """Sum of HW exec times (ns) of the launches in the last kernel() call,
    when KERNEL_TRACE=1 was set and NTFF profiling is available. None otherwise."""
    return _last_exec_ns


def predicted_exec_time_ns():
    """Cost-model (TimelineSim) predicted HW exec time for both launches, ns."""
    from concourse.timeline_sim import TimelineSim
    nc = _build_nc()
    return int(2 * TimelineSim(nc).simulate())


def _run_launch(cols_re, cols_im):
    """cols_re/cols_im: list of 8 planes [4096 f][512 b] f32.
    Returns list of 8 (Fre, Fim) planes [4096 k][512 b]."""
    global _last_exec_ns
    import os
    nc = _build_nc()
    bdc, bm = _make_consts()
    in_maps = []
    for c in range(NCORES):
        in_maps.append({
            "in2": _marshal_in(cols_re[c], cols_im[c]),
            "bdc": bdc, "bm": bm,
        })
    trace = bool(os.environ.get("KERNEL_TRACE"))
    try:
        res = run_bass_kernel_spmd(nc, in_maps, core_ids=list(range(NCORES)),
                                   trace=trace)
    except ModuleNotFoundError:
        # NTFF profiling hook unavailable under this axon client; run untraced.
        res = run_bass_kernel_spmd(nc, in_maps, core_ids=list(range(NCORES)))
    if trace and getattr(res, "exec_time_ns", None) is not None:
        _last_exec_ns = (_last_exec_ns or 0) + res.exec_time_ns
    return [_unmarshal_out(res.results[c]["out2"]) for c in range(NCORES)]


# ---------------------------------------------------------------------------
# public entry point
# ---------------------------------------------------------------------------

def kernel(x: np.ndarray) -> np.ndarray:
    """x: [N, 2] float32 (re, im). Returns FFT(x) as [N, 2] float32."""
    global _last_exec_ns
    _last_exec_ns = None
    x = np.asarray(x)
    Are = np.ascontiguousarray(x[:, 0].reshape(NG, NG))  # [j2g][j1g]
    Aim = np.ascontiguousarray(x[:, 1].reshape(NG, NG))

    # launch 1: FFT over rows (j2g) for each column j1g
    cols_re = [np.ascontiguousarray(Are[:, BPC * c:BPC * (c + 1)]) for c in range(NCORES)]
    cols_im = [np.ascontiguousarray(Aim[:, BPC * c:BPC * (c + 1)]) for c in range(NCORES)]
    l1 = _run_launch(cols_re, cols_im)

    # host: assemble F [k2g][j1g], twiddle, transpose-exchange
    F = np.empty((NG, NG), np.complex64)
    for c in range(NCORES):
        fre, fim = l1[c]
        F[:, BPC * c:BPC * (c + 1)] = fre + 1j * fim
    F *= _global_twiddle()

    # launch 2: FFT over j1g for each row k2g; core d gets rows [512d, 512(d+1))
    cols_re2 = []
    cols_im2 = []
    for d in range(NCORES):
        block = F[BPC * d:BPC * (d + 1), :].T      # [j1g][k2g-local]
        cols_re2.append(np.ascontiguousarray(block.real))
        cols_im2.append(np.ascontiguousarray(block.imag))
    l2 = _run_launch(cols_re2, cols_im2)

    # assemble Xmat [k1g][k2g]; out flat index k = 4096*k1g + k2g
    out = np.empty((NG, NG, 2), np.float32)
    for d in range(NCORES):
        rre, rim = l2[d]
        out[:, BPC * d:BPC * (d + 1), 0] = rre
        out[:, BPC * d:BPC * (d + 1), 1] = rim
    return out.reshape(N, 2)



# revision 28
# speedup vs baseline: 1.8715x; 1.0259x over previous
"""Distributed FFT (N = 2^24 complex points) on 8 Trainium2 NeuronCores.

Four-step (Cooley-Tukey) decomposition N = 4096 x 4096:
  launch 1: per global column j1g, FFT_4096 over j2g      (batch parallel over j1g)
  host:     global twiddle wN^{j1g*k2g} + transpose exchange
  launch 2: per global row k2g, FFT_4096 over j1g         (batch parallel over k2g)

Both launches run the SAME compiled SPMD kernel on all 8 cores: a batch of
512 local FFT_4096 per core. Each FFT_4096 = radix-32 stage (block-diag 4x
packed over the contraction axis, K=128) fused with its inter-stage transpose
(data-stationary matmul: psum[j1, :] += S_slice.T @ [[Wr|Wi],[-Wi|Wr]]),
then a radix-128 stage whose twiddle exp(-2pi i j1 kap2/4096) is folded into
32 per-kap2 weight matrices {Br, Bi, -Bi}. All arithmetic is float32r
TensorE matmuls with fp32 PSUM accumulation; all moving dims are >= 256 for
full-rate f32r.

Local FFT_4096 digits: f = j1 + 128*j2 (j1 in [0,128) fast, j2 in [0,32));
k = kap2 + 32*kap1. Batch b = 128*t + 32*g + s (t chunk of 128, g K-pack
group, s in [0,32)). Host does all layout marshalling (numpy index shuffles);
device sees only contiguous [128, X] DMAs.
"""
import numpy as np

import concourse.mybir as mybir
import concourse.tile as tile
from concourse import bacc
from concourse.bass_utils import run_bass_kernel_spmd

NG = 4096                 # global matrix dimension; N = NG*NG
N = NG * NG
NCORES = 8
BPC = NG // NCORES        # 512 signals per core per launch
import os as _os
NCHUNK = 4                # chunks of 128 signals
_ABLATE = ""              # debug ablations disabled in the shipped kernel

import ml_dtypes
_BF16_NP = ml_dtypes.bfloat16

_F32 = mybir.dt.float32
_BF16 = mybir.dt.bfloat16

# ---------------------------------------------------------------------------
# constants (host-side numpy)
# ---------------------------------------------------------------------------

_consts_cache = None


def _make_consts():
    global _consts_cache
    if _consts_cache is not None:
        return _consts_cache
    j2 = np.arange(32)
    W32 = np.exp(-2j * np.pi * np.outer(j2, j2) / 32)
    I4 = np.eye(4)
    BDr = np.kron(I4, W32.real)
    BDi = np.kron(I4, W32.imag)
    # moving-operand matrices for the fused stageA+transpose matmuls:
    #   psum[j1, 0:128] = Fr, psum[j1, 128:256] = Fi  (accumulated over Sr, Si)
    bdc = np.stack([
        np.concatenate([BDr, BDi], axis=1),     # applied to Sr
        np.concatenate([-BDi, BDr], axis=1),    # applied to Si
    ]).astype(_BF16_NP)                         # [2,128,256]

    j1 = np.arange(128)
    W128 = np.exp(-2j * np.pi * np.outer(j1, j1) / 128)
    bm = np.zeros((32, 3, 128, 128), np.float32)
    for kap2 in range(32):
        B = np.exp(-2j * np.pi * j1 * kap2 / 4096)[:, None] * W128  # [j1][kap1]
        bm[kap2, 0] = B.real
        bm[kap2, 1] = B.imag
        bm[kap2, 2] = -B.imag
    # p-major DRAM layout [p=128, kap2=32, v=3, kap1=128] so the device DMA
    # is one contiguous run per partition (24 KiB in bf16).
    bm = np.ascontiguousarray(bm.transpose(2, 0, 1, 3)).astype(_BF16_NP)
    _consts_cache = (bdc, bm)
    return _consts_cache


_tw_cache = None


def _global_twiddle():
    """exp(-2pi i k2g*j1g / N) as complex64 [NG, NG] (k2g rows)."""
    global _tw_cache
    if _tw_cache is None:
        k = np.arange(NG, dtype=np.float64)
        phase = np.outer(k, k) * (-2.0 * np.pi / N)
        _tw_cache = np.exp(1j * phase).astype(np.complex64)
    return _tw_cache


# ---------------------------------------------------------------------------
# marshalling (host)
# ---------------------------------------------------------------------------

def _marshal_in(Vre, Vim):
    """Vre/Vim: [4096 f][512 b] f32 planes -> in2 [4,2,128,4096] bf16."""
    out = np.empty((NCHUNK, 2, 128, 4096), _BF16_NP)
    for pl, V in ((0, Vre), (1, Vim)):
        V2 = V.reshape(32, 128, 4, 4, 32)      # j2, j1, t, g, s
        out[:, pl] = V2.transpose(2, 3, 0, 4, 1).reshape(4, 128, 4096).astype(_BF16_NP)
    return out


def _unmarshal_out(O):
    """out2 [2,16,2,128,512] bf16 (dims sc,kp,pl,kap1,n2; n2=256u+128c2+4s+g)
    -> (Fre, Fim) planes [4096 k][512 b]."""
    O = np.asarray(O).astype(np.float32)
    O8 = O.reshape(2, 16, 2, 128, 2, 2, 32, 4)  # sc, kp, pl, kap1, u, c2, s, g
    # kap2 = 2*kp+u ; k = 32*kap1 + kap2 ; b = 256*sc + 128*c2 + 32*g + s
    P = np.ascontiguousarray(O8.transpose(2, 3, 1, 4, 0, 5, 7, 6)).reshape(2, 4096, 512)
    return P[0], P[1]


# ---------------------------------------------------------------------------
# device kernel (Bass/Tile), shared by both launches
# ---------------------------------------------------------------------------

_nc_cache = None


def _build_nc():
    global _nc_cache
    if _nc_cache is not None:
        return _nc_cache

    nc = bacc.Bacc(trn_type="TRN2")
    DT = _BF16
    # in layout: [chunk, plane, p = 32g+j2, ff = 128s+j1]
    in_d = nc.dram_tensor("in2", [NCHUNK, 2, 128, 4096], DT, kind="ExternalInput")
    bdc_d = nc.dram_tensor("bdc", [2, 128, 256], DT, kind="ExternalInput")
    bm_d = nc.dram_tensor("bm", [128, 32, 3, 128], DT, kind="ExternalInput")
    # out layout: [superchunk, kap2pair, plane, kap1, n2], n2 = 128*c2 + 4*s + g
    out_d = nc.dram_tensor("out2", [NCHUNK // 2, 16, 2, 128, 512], DT,
                           kind="ExternalOutput")

    with tile.TileContext(nc) as tc:
        with (
            tc.tile_pool(name="consts", bufs=1) as cpool,
            tc.tile_pool(name="inp", bufs=4) as inpool,
            tc.tile_pool(name="tp", bufs=2) as tpool,
            tc.tile_pool(name="outp", bufs=8) as outpool,
            tc.tile_pool(name="pp", bufs=8, space="PSUM") as pp,
        ):
            st = {}

            def load_chunk(t, first=False):
                if t in st or t >= NCHUNK:
                    return
                s = inpool.tile([128, 2, 4096], DT, tag="in")
                # quarter-DMAs so stage A can start on the first piece early;
                # for chunk 0 an even smaller first eighth, with the (tiny)
                # bdc const DMA slotted right after it.
                if first:
                    for q in range(2):
                        qs = slice(512 * q, 512 * q + 512)
                        nc.sync.dma_start(s[:, :, qs],
                                          in_d[t, :, :, qs].rearrange("a p f -> p a f"))
                        if q == 0:
                            nc.sync.dma_start(
                                bdc_t[:], bdc_d.ap().rearrange("a p f -> p a f"))
                    qlist = range(1, 4)
                else:
                    qlist = range(4)
                for q in qlist:
                    qs = slice(1024 * q, 1024 * q + 1024)
                    nc.sync.dma_start(s[:, :, qs],
                                      in_d[t, :, :, qs].rearrange("a p f -> p a f"))
                st[t] = s

            bdc_t = cpool.tile([128, 2, 256], DT, tag="bdc")
            load_chunk(0, first=True)
            load_chunk(1)

            # bm streams in 4 kap2-groups interleaved with chunk prefetches,
            # each arriving just before stage B consumes it.
            bm_t = cpool.tile([128, 32, 3, 128], DT, tag="bm")
            bm_loaded = [False] * 4

            def load_bm_group(g):
                if g >= 4 or bm_loaded[g]:
                    return
                ks = slice(8 * g, 8 * g + 8)
                nc.sync.dma_start(bm_t[:, ks], bm_d[:, ks])
                bm_loaded[g] = True

            load_bm_group(0)

            ncopy = 0  # alternate DVE/ACT for all PSUM evacuations

            def evac(out_ap, in_ap):
                nonlocal ncopy
                if ncopy % 2 == 0:
                    nc.vector.tensor_copy(out_ap, in_ap)
                else:
                    nc.scalar.copy(out_ap, in_ap)
                ncopy += 1

            def make_tt():
                # T for a superchunk: [p=j1][c2][s][plane][g][kap]
                tt = tpool.tile([128, 2, 32, 2, 4, 32], DT, tag="tt")
                ttf = tt.rearrange("p a b c d e -> p (a b c d e)")
                return tt, ttf

            def emit_A_sp(ttf, c2, s, sp):
                # fused stage A + transpose for s-pair sp:
                #   psum[j1, pl*128 + 32g+kap2] += S_sl.T @ bdc[pl-combo]
                bank = pp.tile([128, 512], _F32, tag="ps")
                for e in range(2):
                    sl = 2 * sp + e          # s within chunk
                    ds = slice(128 * sl, 128 * sl + 128)
                    ys = slice(256 * e, 256 * e + 256)
                    nc.tensor.matmul(bank[:, ys], s[:, 0, ds],
                                     bdc_t[:, 0], start=True, stop=False)
                    nc.tensor.matmul(bank[:, ys], s[:, 1, ds],
                                     bdc_t[:, 1], start=False, stop=True)
                off = (c2 * 32 + 2 * sp) * 256
                evac(ttf[:, off:off + 512], bank[:])

            def emit_B_kp(tt, sc, kp):
                # stage B: radix-128, per-kap2 twiddled weights, N=256
                ot = outpool.tile([128, 2, 512], DT, tag="out")
                yr = pp.tile([128, 512], _F32, tag="ps")
                yi = pp.tile([128, 512], _F32, tag="ps")
                for u in range(2):
                    kap2 = 2 * kp + u
                    ys = slice(256 * u, 256 * u + 256)
                    trs = tt[:, :, :, 0, :, kap2]
                    tis = tt[:, :, :, 1, :, kap2]
                    br = bm_t[:, kap2, 0]
                    bi = bm_t[:, kap2, 1]
                    bni = bm_t[:, kap2, 2]
                    nc.tensor.matmul(yr[:, ys], br, trs, start=True, stop=False)
                    nc.tensor.matmul(yi[:, ys], br, tis, start=True, stop=False)
                    nc.tensor.matmul(yr[:, ys], bni, tis, start=False, stop=True)
                    nc.tensor.matmul(yi[:, ys], bi, trs, start=False, stop=True)
                evac(ot[:, 0], yr[:])
                evac(ot[:, 1], yi[:])
                nc.sync.dma_start(
                    out_d[sc, kp].rearrange("a p f -> p a f"), ot[:])

            # Emission order keeps PE busy across every A->B boundary:
            #   A(t0) A(t1) [A(t2)x2 ; B(sc0)]x8 [A(t3)x2 ; B(sc0)]x8 B(sc1)
            # B(sc0) is spread 2-sp:1-kp across A(t2)+A(t3), so tt1's last
            # evacuations land while B(sc0)'s tail kps still run on PE.
            tt0, ttf0 = make_tt()
            s0 = st.pop(0)
            for sp in range(16):
                emit_A_sp(ttf0, 0, s0, sp)
            s1 = st.pop(1)
            for sp in range(16):
                emit_A_sp(ttf0, 1, s1, sp)

            tt1, ttf1 = make_tt()
            load_chunk(2)
            load_bm_group(1)
            load_bm_group(2)
            load_bm_group(3)
            load_chunk(3)
            s2 = st.pop(2)
            for i in range(8):
                emit_A_sp(ttf1, 0, s2, 2 * i)
                emit_A_sp(ttf1, 0, s2, 2 * i + 1)
                emit_B_kp(tt0, 0, i)
            s3 = st.pop(3)
            for j in range(8):
                emit_A_sp(ttf1, 1, s3, 2 * j)
                emit_A_sp(ttf1, 1, s3, 2 * j + 1)
                emit_B_kp(tt0, 0, 8 + j)
            for kp in range(16):
                emit_B_kp(tt1, 1, kp)

    nc.finalize()
    _nc_cache = nc
    return nc


# ---------------------------------------------------------------------------
# launch helper
# ---------------------------------------------------------------------------

_last_exec_ns = None


def last_exec_time_ns():
    """Sum of HW exec times (ns) of the launches in the last kernel() call,
    when KERNEL_TRACE=1 was set and NTFF profiling is available. None otherwise."""
    return _last_exec_ns


def predicted_exec_time_ns():
    """Cost-model (TimelineSim) predicted HW exec time for both launches, ns."""
    from concourse.timeline_sim import TimelineSim
    nc = _build_nc()
    return int(2 * TimelineSim(nc).simulate())


def _run_launch(cols_re, cols_im):
    """cols_re/cols_im: list of 8 planes [4096 f][512 b] f32.
    Returns list of 8 (Fre, Fim) planes [4096 k][512 b]."""
    global _last_exec_ns
    import os
    nc = _build_nc()
    bdc, bm = _make_consts()
    in_maps = []
    for c in range(NCORES):
        in_maps.append({
            "in2": _marshal_in(cols_re[c], cols_im[c]),
            "bdc": bdc, "bm": bm,
        })
    trace = bool(os.environ.get("KERNEL_TRACE"))
    try:
        res = run_bass_kernel_spmd(nc, in_maps, core_ids=list(range(NCORES)),
                                   trace=trace)
    except ModuleNotFoundError:
        # NTFF profiling hook unavailable under this axon client; run untraced.
        res = run_bass_kernel_spmd(nc, in_maps, core_ids=list(range(NCORES)))
    if trace and getattr(res, "exec_time_ns", None) is not None:
        _last_exec_ns = (_last_exec_ns or 0) + res.exec_time_ns
    return [_unmarshal_out(res.results[c]["out2"]) for c in range(NCORES)]


# ---------------------------------------------------------------------------
# public entry point
# ---------------------------------------------------------------------------

def kernel(x: np.ndarray) -> np.ndarray:
    """x: [N, 2] float32 (re, im). Returns FFT(x) as [N, 2] float32."""
    global _last_exec_ns
    _last_exec_ns = None
    x = np.asarray(x)
    Are = np.ascontiguousarray(x[:, 0].reshape(NG, NG))  # [j2g][j1g]
    Aim = np.ascontiguousarray(x[:, 1].reshape(NG, NG))

    # launch 1: FFT over rows (j2g) for each column j1g
    cols_re = [np.ascontiguousarray(Are[:, BPC * c:BPC * (c + 1)]) for c in range(NCORES)]
    cols_im = [np.ascontiguousarray(Aim[:, BPC * c:BPC * (c + 1)]) for c in range(NCORES)]
    l1 = _run_launch(cols_re, cols_im)

    # host: assemble F [k2g][j1g], twiddle, transpose-exchange
    F = np.empty((NG, NG), np.complex64)
    for c in range(NCORES):
        fre, fim = l1[c]
        F[:, BPC * c:BPC * (c + 1)] = fre + 1j * fim
    F *= _global_twiddle()

    # launch 2: FFT over j1g for each row k2g; core d gets rows [512d, 512(d+1))
    cols_re2 = []
    cols_im2 = []
    for d in range(NCORES):
        block = F[BPC * d:BPC * (d + 1), :].T      # [j1g][k2g-local]
        cols_re2.append(np.ascontiguousarray(block.real))
        cols_im2.append(np.ascontiguousarray(block.imag))
    l2 = _run_launch(cols_re2, cols_im2)

    # assemble Xmat [k1g][k2g]; out flat index k = 4096*k1g + k2g
    out = np.empty((NG, NG, 2), np.float32)
    for d in range(NCORES):
        rre, rim = l2[d]
        out[:, BPC * d:BPC * (d + 1), 0] = rre
        out[:, BPC * d:BPC * (d + 1), 1] = rim
    return out.reshape(N, 2)



# revision 30
# speedup vs baseline: 1.9497x; 1.0418x over previous
"""Distributed FFT (N = 2^24 complex points) on 8 Trainium2 NeuronCores.

Four-step (Cooley-Tukey) decomposition N = 4096 x 4096:
  launch 1: per global column j1g, FFT_4096 over j2g      (batch parallel over j1g)
  host:     global twiddle wN^{j1g*k2g} + transpose exchange
  launch 2: per global row k2g, FFT_4096 over j1g         (batch parallel over k2g)

Both launches run the SAME compiled SPMD kernel on all 8 cores: a batch of
512 local FFT_4096 per core. Each FFT_4096 = radix-32 stage (block-diag 4x
packed over the contraction axis, K=128) fused with its inter-stage transpose
(data-stationary matmul: psum[j1, :] += S_slice.T @ [[Wr|Wi],[-Wi|Wr]]),
then a radix-128 stage whose twiddle exp(-2pi i j1 kap2/4096) is folded into
32 per-kap2 weight matrices {Br, Bi, -Bi}. All arithmetic is float32r
TensorE matmuls with fp32 PSUM accumulation; all moving dims are >= 256 for
full-rate f32r.

Local FFT_4096 digits: f = j1 + 128*j2 (j1 in [0,128) fast, j2 in [0,32));
k = kap2 + 32*kap1. Batch b = 128*t + 32*g + s (t chunk of 128, g K-pack
group, s in [0,32)). Host does all layout marshalling (numpy index shuffles);
device sees only contiguous [128, X] DMAs.
"""
import numpy as np

import concourse.mybir as mybir
import concourse.tile as tile
from concourse import bacc
from concourse.bass_utils import run_bass_kernel_spmd

NG = 4096                 # global matrix dimension; N = NG*NG
N = NG * NG
NCORES = 8
BPC = NG // NCORES        # 512 signals per core per launch
import os as _os
NCHUNK = 4                # chunks of 128 signals
_ABLATE = ""              # debug ablations disabled in the shipped kernel

import ml_dtypes
_BF16_NP = ml_dtypes.bfloat16

_F32 = mybir.dt.float32
_BF16 = mybir.dt.bfloat16

# ---------------------------------------------------------------------------
# constants (host-side numpy)
# ---------------------------------------------------------------------------

_consts_cache = None


def _make_consts():
    global _consts_cache
    if _consts_cache is not None:
        return _consts_cache
    j2 = np.arange(32)
    W32 = np.exp(-2j * np.pi * np.outer(j2, j2) / 32)
    I4 = np.eye(4)
    BDr = np.kron(I4, W32.real)
    BDi = np.kron(I4, W32.imag)
    # moving-operand matrices for the fused stageA+transpose matmuls:
    #   psum[j1, 0:128] = Fr, psum[j1, 128:256] = Fi  (accumulated over Sr, Si)
    bdc = np.stack([
        np.concatenate([BDr, BDi], axis=1),     # applied to Sr
        np.concatenate([-BDi, BDr], axis=1),    # applied to Si
    ]).astype(_BF16_NP)                         # [2,128,256]

    j1 = np.arange(128)
    W128 = np.exp(-2j * np.pi * np.outer(j1, j1) / 128)
    bm = np.zeros((32, 2, 128, 128), np.float32)
    for kap2 in range(32):
        B = np.exp(-2j * np.pi * j1 * kap2 / 4096)[:, None] * W128  # [j1][kap1]
        bm[kap2, 0] = B.real
        bm[kap2, 1] = B.imag
    # p-major DRAM layout [p=128, kap2=32, v=2, kap1=128] so the device DMA
    # is one contiguous run per partition (16 KiB in bf16).
    bm = np.ascontiguousarray(bm.transpose(2, 0, 1, 3)).astype(_BF16_NP)
    _consts_cache = (bdc, bm)
    return _consts_cache


_tw_cache = None


def _global_twiddle():
    """exp(-2pi i k2g*j1g / N) as complex64 [NG, NG] (k2g rows)."""
    global _tw_cache
    if _tw_cache is None:
        k = np.arange(NG, dtype=np.float64)
        phase = np.outer(k, k) * (-2.0 * np.pi / N)
        _tw_cache = np.exp(1j * phase).astype(np.complex64)
    return _tw_cache


# ---------------------------------------------------------------------------
# marshalling (host)
# ---------------------------------------------------------------------------

def _marshal_in(Vre, Vim):
    """Vre/Vim: [4096 f][512 b] f32 planes -> in2 [4,2,128,4096] bf16."""
    out = np.empty((NCHUNK, 2, 128, 4096), _BF16_NP)
    for pl, V in ((0, Vre), (1, Vim)):
        V2 = V.reshape(32, 128, 4, 4, 32)      # j2, j1, t, g, s
        out[:, pl] = V2.transpose(2, 3, 0, 4, 1).reshape(4, 128, 4096).astype(_BF16_NP)
    return out


def _unmarshal_out(O):
    """out2 [2,16,2,128,512] bf16 (dims sc,kp,pl,kap1,n2; n2=256u+128c2+4s+g)
    -> (Fre, Fim) planes [4096 k][512 b]."""
    O = np.asarray(O).astype(np.float32)
    O8 = O.reshape(2, 16, 2, 128, 2, 2, 32, 4)  # sc, kp, pl, kap1, u, c2, s, g
    # kap2 = 2*kp+u ; k = 32*kap1 + kap2 ; b = 256*sc + 128*c2 + 32*g + s
    P = np.ascontiguousarray(O8.transpose(2, 3, 1, 4, 0, 5, 7, 6)).reshape(2, 4096, 512)
    return P[0], P[1]


# ---------------------------------------------------------------------------
# device kernel (Bass/Tile), shared by both launches
# ---------------------------------------------------------------------------

_nc_cache = None


def _build_nc():
    global _nc_cache
    if _nc_cache is not None:
        return _nc_cache

    nc = bacc.Bacc(trn_type="TRN2")
    DT = _BF16
    # in layout: [chunk, plane, p = 32g+j2, ff = 128s+j1]
    in_d = nc.dram_tensor("in2", [NCHUNK, 2, 128, 4096], DT, kind="ExternalInput")
    bdc_d = nc.dram_tensor("bdc", [2, 128, 256], DT, kind="ExternalInput")
    bm_d = nc.dram_tensor("bm", [128, 32, 2, 128], DT, kind="ExternalInput")
    # out layout: [superchunk, kap2pair, plane, kap1, n2], n2 = 128*c2 + 4*s + g
    out_d = nc.dram_tensor("out2", [NCHUNK // 2, 16, 2, 128, 512], DT,
                           kind="ExternalOutput")

    with tile.TileContext(nc) as tc:
        with (
            tc.tile_pool(name="consts", bufs=1) as cpool,
            tc.tile_pool(name="inp", bufs=4) as inpool,
            tc.tile_pool(name="tp", bufs=2) as tpool,
            tc.tile_pool(name="outp", bufs=8) as outpool,
            tc.tile_pool(name="pp", bufs=8, space="PSUM") as pp,
        ):
            st = {}

            def load_chunk(t, first=False):
                if t in st or t >= NCHUNK:
                    return
                s = inpool.tile([128, 2, 4096], DT, tag="in")
                # quarter-DMAs so stage A can start on the first piece early;
                # for chunk 0 an even smaller first eighth, with the (tiny)
                # bdc const DMA slotted right after it.
                if first:
                    # bdc rides the Pool/SWDGE path in parallel with the
                    # HWDGE input pieces; fine-grained leading pieces so the
                    # first matmul starts as early as possible
                    nc.gpsimd.dma_start(bdc_t[:],
                                        bdc_d.ap().rearrange("a p f -> p a f"))
                    bounds = [0, 256, 512, 1024, 2048, 3072, 4096]
                    for i in range(len(bounds) - 1):
                        qs = slice(bounds[i], bounds[i + 1])
                        nc.sync.dma_start(s[:, :, qs],
                                          in_d[t, :, :, qs].rearrange("a p f -> p a f"))
                else:
                    for q in range(4):
                        qs = slice(1024 * q, 1024 * q + 1024)
                        nc.sync.dma_start(s[:, :, qs],
                                          in_d[t, :, :, qs].rearrange("a p f -> p a f"))
                st[t] = s

            bdc_t = cpool.tile([128, 2, 256], DT, tag="bdc")
            load_chunk(0, first=True)
            load_chunk(1)

            # bm streams in 4 kap2-groups interleaved with chunk prefetches,
            # each arriving just before stage B consumes it. Only {br, bi}
            # come over DMA; the -bi slab is synthesized in SBUF by one DVE
            # negate per group (PSUM-operand rules forbid p1-p2 evacuation).
            bm_t = cpool.tile([128, 32, 2, 128], DT, tag="bm")
            bnm_t = cpool.tile([128, 32, 128], DT, tag="bnm")
            bm_loaded = [False] * 4

            def load_bm_group(g):
                if g >= 4 or bm_loaded[g]:
                    return
                ks = slice(8 * g, 8 * g + 8)
                nc.sync.dma_start(bm_t[:, ks], bm_d[:, ks])
                nc.vector.tensor_scalar_mul(bnm_t[:, ks], bm_t[:, ks, 1], -1.0)
                bm_loaded[g] = True

            load_bm_group(0)

            ncopy = 0  # alternate DVE/ACT for all PSUM evacuations

            def evac(out_ap, in_ap):
                nonlocal ncopy
                if ncopy % 2 == 0:
                    nc.vector.tensor_copy(out_ap, in_ap)
                else:
                    nc.scalar.copy(out_ap, in_ap)
                ncopy += 1

            def make_tt():
                # T for a superchunk: [p=j1][c2][s][plane][g][kap]
                tt = tpool.tile([128, 2, 32, 2, 4, 32], DT, tag="tt")
                ttf = tt.rearrange("p a b c d e -> p (a b c d e)")
                return tt, ttf

            def emit_A_sp(ttf, c2, s, sp):
                # fused stage A + transpose for s-pair sp:
                #   psum[j1, pl*128 + 32g+kap2] += S_sl.T @ bdc[pl-combo]
                bank = pp.tile([128, 512], _F32, tag="ps")
                for e in range(2):
                    sl = 2 * sp + e          # s within chunk
                    ds = slice(128 * sl, 128 * sl + 128)
                    ys = slice(256 * e, 256 * e + 256)
                    nc.tensor.matmul(bank[:, ys], s[:, 0, ds],
                                     bdc_t[:, 0], start=True, stop=False)
                    nc.tensor.matmul(bank[:, ys], s[:, 1, ds],
                                     bdc_t[:, 1], start=False, stop=True)
                off = (c2 * 32 + 2 * sp) * 256
                evac(ttf[:, off:off + 512], bank[:])

            def emit_B_kp(tt, sc, kp):
                # stage B: radix-128, per-kap2 twiddled weights, N=256
                ot = outpool.tile([128, 2, 512], DT, tag="out")
                yr = pp.tile([128, 512], _F32, tag="ps")
                yi = pp.tile([128, 512], _F32, tag="ps")
                for u in range(2):
                    kap2 = 2 * kp + u
                    ys = slice(256 * u, 256 * u + 256)
                    trs = tt[:, :, :, 0, :, kap2]
                    tis = tt[:, :, :, 1, :, kap2]
                    br = bm_t[:, kap2, 0]
                    bi = bm_t[:, kap2, 1]
                    bni = bnm_t[:, kap2]
                    nc.tensor.matmul(yr[:, ys], br, trs, start=True, stop=False)
                    nc.tensor.matmul(yi[:, ys], br, tis, start=True, stop=False)
                    nc.tensor.matmul(yr[:, ys], bni, tis, start=False, stop=True)
                    nc.tensor.matmul(yi[:, ys], bi, trs, start=False, stop=True)
                evac(ot[:, 0], yr[:])
                evac(ot[:, 1], yi[:])
                nc.sync.dma_start(
                    out_d[sc, kp].rearrange("a p f -> p a f"), ot[:])

            # Emission order keeps PE busy across every A->B boundary:
            #   A(t0) A(t1) [A(t2)x2 ; B(sc0)]x8 [A(t3)x2 ; B(sc0)]x8 B(sc1)
            # B(sc0) is spread 2-sp:1-kp across A(t2)+A(t3), so tt1's last
            # evacuations land while B(sc0)'s tail kps still run on PE.
            tt0, ttf0 = make_tt()
            s0 = st.pop(0)
            for sp in range(16):
                emit_A_sp(ttf0, 0, s0, sp)
            s1 = st.pop(1)
            for sp in range(16):
                emit_A_sp(ttf0, 1, s1, sp)

            tt1, ttf1 = make_tt()
            load_chunk(2)
            load_bm_group(1)
            load_bm_group(2)
            load_bm_group(3)
            load_chunk(3)
            s2 = st.pop(2)
            for i in range(8):
                emit_A_sp(ttf1, 0, s2, 2 * i)
                emit_A_sp(ttf1, 0, s2, 2 * i + 1)
                emit_B_kp(tt0, 0, i)
            s3 = st.pop(3)
            for j in range(8):
                emit_A_sp(ttf1, 1, s3, 2 * j)
                emit_A_sp(ttf1, 1, s3, 2 * j + 1)
                emit_B_kp(tt0, 0, 8 + j)
            for kp in range(16):
                emit_B_kp(tt1, 1, kp)

    nc.finalize()
    _nc_cache = nc
    return nc


# ---------------------------------------------------------------------------
# launch helper
# ---------------------------------------------------------------------------

_last_exec_ns = None


def last_exec_time_ns():
    """Sum of HW exec times (ns) of the launches in the last kernel() call,
    when KERNEL_TRACE=1 was set and NTFF profiling is available. None otherwise."""
    return _last_exec_ns


def predicted_exec_time_ns():
    """Cost-model (TimelineSim) predicted HW exec time for both launches, ns."""
    from concourse.timeline_sim import TimelineSim
    nc = _build_nc()
    return int(2 * TimelineSim(nc).simulate())


def _run_launch(cols_re, cols_im):
    """cols_re/cols_im: list of 8 planes [4096 f][512 b] f32.
    Returns list of 8 (Fre, Fim) planes [4096 k][512 b]."""
    global _last_exec_ns
    import os
    nc = _build_nc()
    bdc, bm = _make_consts()
    in_maps = []
    for c in range(NCORES):
        in_maps.append({
            "in2": _marshal_in(cols_re[c], cols_im[c]),
            "bdc": bdc, "bm": bm,
        })
    trace = bool(os.environ.get("KERNEL_TRACE"))
    try:
        res = run_bass_kernel_spmd(nc, in_maps, core_ids=list(range(NCORES)),
                                   trace=trace)
    except ModuleNotFoundError:
        # NTFF profiling hook unavailable under this axon client; run untraced.
        res = run_bass_kernel_spmd(nc, in_maps, core_ids=list(range(NCORES)))
    if trace and getattr(res, "exec_time_ns", None) is not None:
        _last_exec_ns = (_last_exec_ns or 0) + res.exec_time_ns
    return [_unmarshal_out(res.results[c]["out2"]) for c in range(NCORES)]


# ---------------------------------------------------------------------------
# public entry point
# ---------------------------------------------------------------------------

def kernel(x: np.ndarray) -> np.ndarray:
    """x: [N, 2] float32 (re, im). Returns FFT(x) as [N, 2] float32."""
    global _last_exec_ns
    _last_exec_ns = None
    x = np.asarray(x)
    Are = np.ascontiguousarray(x[:, 0].reshape(NG, NG))  # [j2g][j1g]
    Aim = np.ascontiguousarray(x[:, 1].reshape(NG, NG))

    # launch 1: FFT over rows (j2g) for each column j1g
    cols_re = [np.ascontiguousarray(Are[:, BPC * c:BPC * (c + 1)]) for c in range(NCORES)]
    cols_im = [np.ascontiguousarray(Aim[:, BPC * c:BPC * (c + 1)]) for c in range(NCORES)]
    l1 = _run_launch(cols_re, cols_im)

    # host: assemble F [k2g][j1g], twiddle, transpose-exchange
    F = np.empty((NG, NG), np.complex64)
    for c in range(NCORES):
        fre, fim = l1[c]
        F[:, BPC * c:BPC * (c + 1)] = fre + 1j * fim
    F *= _global_twiddle()

    # launch 2: FFT over j1g for each row k2g; core d gets rows [512d, 512(d+1))
    cols_re2 = []
    cols_im2 = []
    for d in range(NCORES):
        block = F[BPC * d:BPC * (d + 1), :].T      # [j1g][k2g-local]
        cols_re2.append(np.ascontiguousarray(block.real))
        cols_im2.append(np.ascontiguousarray(block.imag))
    l2 = _run_launch(cols_re2, cols_im2)

    # assemble Xmat [k1g][k2g]; out flat index k = 4096*k1g + k2g
    out = np.empty((NG, NG, 2), np.float32)
    for d in range(NCORES):
        rre, rim = l2[d]
        out[:, BPC * d:BPC * (d + 1), 0] = rre
        out[:, BPC * d:BPC * (d + 1), 1] = rim
    return out.reshape(N, 2)



# revision 33
# speedup vs baseline: 1.9679x; 1.0094x over previous
"""Distributed FFT (N = 2^24 complex points) on 8 Trainium2 NeuronCores.

Four-step (Cooley-Tukey) decomposition N = 4096 x 4096:
  launch 1: per global column j1g, FFT_4096 over j2g      (batch parallel over j1g)
  host:     global twiddle wN^{j1g*k2g} + transpose exchange
  launch 2: per global row k2g, FFT_4096 over j1g         (batch parallel over k2g)

Both launches run the SAME compiled SPMD kernel on all 8 cores: a batch of
512 local FFT_4096 per core. Each FFT_4096 = radix-32 stage (block-diag 4x
packed over the contraction axis, K=128) fused with its inter-stage transpose
(data-stationary matmul: psum[j1, :] += S_slice.T @ [[Wr|Wi],[-Wi|Wr]]),
then a radix-128 stage whose twiddle exp(-2pi i j1 kap2/4096) is folded into
32 per-kap2 weight matrices {Br, Bi, -Bi}. All arithmetic is float32r
TensorE matmuls with fp32 PSUM accumulation; all moving dims are >= 256 for
full-rate f32r.

Local FFT_4096 digits: f = j1 + 128*j2 (j1 in [0,128) fast, j2 in [0,32));
k = kap2 + 32*kap1. Batch b = 128*t + 32*g + s (t chunk of 128, g K-pack
group, s in [0,32)). Host does all layout marshalling (numpy index shuffles);
device sees only contiguous [128, X] DMAs.
"""
import numpy as np

import concourse.mybir as mybir
import concourse.tile as tile
from concourse import bacc
from concourse.bass_utils import run_bass_kernel_spmd

NG = 4096                 # global matrix dimension; N = NG*NG
N = NG * NG
NCORES = 8
BPC = NG // NCORES        # 512 signals per core per launch
import os as _os
NCHUNK = 4                # chunks of 128 signals
_ABLATE = ""              # debug ablations disabled in the shipped kernel

import ml_dtypes
_BF16_NP = ml_dtypes.bfloat16

_F32 = mybir.dt.float32
_BF16 = mybir.dt.bfloat16

# ---------------------------------------------------------------------------
# constants (host-side numpy)
# ---------------------------------------------------------------------------

_consts_cache = None


def _make_consts():
    global _consts_cache
    if _consts_cache is not None:
        return _consts_cache
    j2 = np.arange(32)
    W32 = np.exp(-2j * np.pi * np.outer(j2, j2) / 32)
    I4 = np.eye(4)
    BDr = np.kron(I4, W32.real)
    BDi = np.kron(I4, W32.imag)
    # moving-operand matrices for the fused stageA+transpose matmuls:
    #   psum[j1, 0:128] = Fr, psum[j1, 128:256] = Fi  (accumulated over Sr, Si)
    bdc = np.stack([
        np.concatenate([BDr, BDi], axis=1),     # applied to Sr
        np.concatenate([-BDi, BDr], axis=1),    # applied to Si
    ]).astype(_BF16_NP)                         # [2,128,256]

    j1 = np.arange(128)
    W128 = np.exp(-2j * np.pi * np.outer(j1, j1) / 128)
    bm = np.zeros((32, 2, 128, 128), np.float32)
    for kap2 in range(32):
        B = np.exp(-2j * np.pi * j1 * kap2 / 4096)[:, None] * W128  # [j1][kap1]
        bm[kap2, 0] = B.real
        bm[kap2, 1] = B.imag
    # p-major DRAM layout [p=128, kap2=32, v=2, kap1=128] so the device DMA
    # is one contiguous run per partition (16 KiB in bf16).
    bm = np.ascontiguousarray(bm.transpose(2, 0, 1, 3)).astype(_BF16_NP)
    _consts_cache = (bdc, bm)
    return _consts_cache


_tw_cache = None


def _global_twiddle():
    """exp(-2pi i k2g*j1g / N) as complex64 [NG, NG] (k2g rows)."""
    global _tw_cache
    if _tw_cache is None:
        k = np.arange(NG, dtype=np.float64)
        phase = np.outer(k, k) * (-2.0 * np.pi / N)
        _tw_cache = np.exp(1j * phase).astype(np.complex64)
    return _tw_cache


# ---------------------------------------------------------------------------
# marshalling (host)
# ---------------------------------------------------------------------------

def _marshal_in(Vre, Vim):
    """Vre/Vim: [4096 f][512 b] f32 planes -> in2 [4,2,128,4096] bf16."""
    out = np.empty((NCHUNK, 2, 128, 4096), _BF16_NP)
    for pl, V in ((0, Vre), (1, Vim)):
        V2 = V.reshape(32, 128, 4, 4, 32)      # j2, j1, t, g, s
        out[:, pl] = V2.transpose(2, 3, 0, 4, 1).reshape(4, 128, 4096).astype(_BF16_NP)
    return out


def _unmarshal_out(O):
    """out2 [2,16,2,128,512] bf16 (dims sc,kp,pl,kap1,n2; n2=256u+128c2+4s+g)
    -> (Fre, Fim) planes [4096 k][512 b]."""
    O = np.asarray(O).astype(np.float32)
    O8 = O.reshape(2, 16, 2, 128, 2, 2, 32, 4)  # sc, kp, pl, kap1, u, c2, s, g
    # kap2 = 2*kp+u ; k = 32*kap1 + kap2 ; b = 256*sc + 128*c2 + 32*g + s
    P = np.ascontiguousarray(O8.transpose(2, 3, 1, 4, 0, 5, 7, 6)).reshape(2, 4096, 512)
    return P[0], P[1]


# ---------------------------------------------------------------------------
# device kernel (Bass/Tile), shared by both launches
# ---------------------------------------------------------------------------

_nc_cache = None


def _build_nc():
    global _nc_cache
    if _nc_cache is not None:
        return _nc_cache

    nc = bacc.Bacc(trn_type="TRN2")
    DT = _BF16
    # in layout: [chunk, plane, p = 32g+j2, ff = 128s+j1]
    in_d = nc.dram_tensor("in2", [NCHUNK, 2, 128, 4096], DT, kind="ExternalInput")
    bdc_d = nc.dram_tensor("bdc", [2, 128, 256], DT, kind="ExternalInput")
    bm_d = nc.dram_tensor("bm", [128, 32, 2, 128], DT, kind="ExternalInput")
    # out layout: [superchunk, kap2pair, plane, kap1, n2], n2 = 128*c2 + 4*s + g
    out_d = nc.dram_tensor("out2", [NCHUNK // 2, 16, 2, 128, 512], DT,
                           kind="ExternalOutput")

    with tile.TileContext(nc) as tc:
        with (
            tc.tile_pool(name="consts", bufs=1) as cpool,
            tc.tile_pool(name="inp", bufs=4) as inpool,
            tc.tile_pool(name="tp", bufs=2) as tpool,
            tc.tile_pool(name="outp", bufs=8) as outpool,
            tc.tile_pool(name="pp", bufs=8, space="PSUM") as pp,
        ):
            st = {}

            def load_chunk(t, first=False):
                if t in st or t >= NCHUNK:
                    return
                s = inpool.tile([128, 2, 4096], DT, tag="in")
                # quarter-DMAs so stage A can start on the first piece early;
                # for chunk 0 an even smaller first eighth, with the (tiny)
                # bdc const DMA slotted right after it.
                if first:
                    # bdc rides the Pool/SWDGE path in parallel with the
                    # HWDGE input pieces; fine-grained leading pieces so the
                    # first matmul starts as early as possible
                    nc.gpsimd.dma_start(bdc_t[:],
                                        bdc_d.ap().rearrange("a p f -> p a f"))
                    bounds = [0, 384, 768, 1152, 1664, 2176, 2688, 3200, 3712, 4096]
                    for i in range(len(bounds) - 1):
                        qs = slice(bounds[i], bounds[i + 1])
                        nc.sync.dma_start(s[:, :, qs],
                                          in_d[t, :, :, qs].rearrange("a p f -> p a f"))
                else:
                    for q in range(4):
                        qs = slice(1024 * q, 1024 * q + 1024)
                        nc.sync.dma_start(s[:, :, qs],
                                          in_d[t, :, :, qs].rearrange("a p f -> p a f"))
                st[t] = s

            bdc_t = cpool.tile([128, 2, 256], DT, tag="bdc")
            load_chunk(0, first=True)
            load_chunk(1)

            # bm streams in 4 kap2-groups interleaved with chunk prefetches,
            # each arriving just before stage B consumes it. Only {br, bi}
            # come over DMA; the -bi slab is synthesized in SBUF by one DVE
            # negate per group (PSUM-operand rules forbid p1-p2 evacuation).
            bm_t = cpool.tile([128, 32, 2, 128], DT, tag="bm")
            bnm_t = cpool.tile([128, 32, 128], DT, tag="bnm")
            bm_loaded = [False] * 4

            def load_bm_group(g):
                if g >= 4 or bm_loaded[g]:
                    return
                ks = slice(8 * g, 8 * g + 8)
                nc.sync.dma_start(bm_t[:, ks], bm_d[:, ks])
                nc.vector.tensor_scalar_mul(bnm_t[:, ks], bm_t[:, ks, 1], -1.0)
                bm_loaded[g] = True

            load_bm_group(0)

            ncopy = 0  # alternate DVE/ACT for all PSUM evacuations

            def evac(out_ap, in_ap):
                nonlocal ncopy
                if ncopy % 2 == 0:
                    nc.vector.tensor_copy(out_ap, in_ap)
                else:
                    nc.scalar.copy(out_ap, in_ap)
                ncopy += 1

            def make_tt():
                # T for a superchunk: [p=j1][c2][s][plane][g][kap]
                tt = tpool.tile([128, 2, 32, 2, 4, 32], DT, tag="tt")
                ttf = tt.rearrange("p a b c d e -> p (a b c d e)")
                return tt, ttf

            def emit_A_sp(ttf, c2, s, sp):
                # fused stage A + transpose for s-pair sp:
                #   psum[j1, pl*128 + 32g+kap2] += S_sl.T @ bdc[pl-combo]
                bank = pp.tile([128, 512], _F32, tag="ps")
                for e in range(2):
                    sl = 2 * sp + e          # s within chunk
                    ds = slice(128 * sl, 128 * sl + 128)
                    ys = slice(256 * e, 256 * e + 256)
                    nc.tensor.matmul(bank[:, ys], s[:, 0, ds],
                                     bdc_t[:, 0], start=True, stop=False)
                    nc.tensor.matmul(bank[:, ys], s[:, 1, ds],
                                     bdc_t[:, 1], start=False, stop=True)
                off = (c2 * 32 + 2 * sp) * 256
                evac(ttf[:, off:off + 512], bank[:])

            def emit_B_kp(tt, sc, kp):
                # stage B: radix-128, per-kap2 twiddled weights, N=256
                ot = outpool.tile([128, 2, 512], DT, tag="out")
                yr = pp.tile([128, 512], _F32, tag="ps")
                yi = pp.tile([128, 512], _F32, tag="ps")
                for u in range(2):
                    kap2 = 2 * kp + u
                    ys = slice(256 * u, 256 * u + 256)
                    trs = tt[:, :, :, 0, :, kap2]
                    tis = tt[:, :, :, 1, :, kap2]
                    br = bm_t[:, kap2, 0]
                    bi = bm_t[:, kap2, 1]
                    bni = bnm_t[:, kap2]
                    nc.tensor.matmul(yr[:, ys], br, trs, start=True, stop=False)
                    nc.tensor.matmul(yi[:, ys], br, tis, start=True, stop=False)
                    nc.tensor.matmul(yr[:, ys], bni, tis, start=False, stop=True)
                    nc.tensor.matmul(yi[:, ys], bi, trs, start=False, stop=True)
                evac(ot[:, 0], yr[:])
                evac(ot[:, 1], yi[:])
                nc.sync.dma_start(
                    out_d[sc, kp].rearrange("a p f -> p a f"), ot[:])

            # Emission order keeps PE busy across every A->B boundary:
            #   A(t0) A(t1) [A(t2)x2 ; B(sc0)]x8 [A(t3)x2 ; B(sc0)]x8 B(sc1)
            # B(sc0) is spread 2-sp:1-kp across A(t2)+A(t3), so tt1's last
            # evacuations land while B(sc0)'s tail kps still run on PE.
            tt0, ttf0 = make_tt()
            s0 = st.pop(0)
            for sp in range(16):
                emit_A_sp(ttf0, 0, s0, sp)
            s1 = st.pop(1)
            for sp in range(16):
                emit_A_sp(ttf0, 1, s1, sp)

            tt1, ttf1 = make_tt()
            load_chunk(2)
            load_bm_group(1)
            load_bm_group(2)
            load_bm_group(3)
            load_chunk(3)
            s2 = st.pop(2)
            for i in range(8):
                emit_A_sp(ttf1, 0, s2, 2 * i)
                emit_A_sp(ttf1, 0, s2, 2 * i + 1)
                emit_B_kp(tt0, 0, i)
            s3 = st.pop(3)
            for j in range(8):
                emit_A_sp(ttf1, 1, s3, 2 * j)
                emit_A_sp(ttf1, 1, s3, 2 * j + 1)
                emit_B_kp(tt0, 0, 8 + j)
            for kp in range(16):
                emit_B_kp(tt1, 1, kp)


    nc.finalize()
    _nc_cache = nc
    return nc


# ---------------------------------------------------------------------------
# launch helper
# ---------------------------------------------------------------------------

_last_exec_ns = None


def last_exec_time_ns():
    """Sum of HW exec times (ns) of the launches in the last kernel() call,
    when KERNEL_TRACE=1 was set and NTFF profiling is available. None otherwise."""
    return _last_exec_ns


def predicted_exec_time_ns():
    """Cost-model (TimelineSim) predicted HW exec time for both launches, ns."""
    from concourse.timeline_sim import TimelineSim
    nc = _build_nc()
    return int(2 * TimelineSim(nc).simulate())


def _run_launch(cols_re, cols_im):
    """cols_re/cols_im: list of 8 planes [4096 f][512 b] f32.
    Returns list of 8 (Fre, Fim) planes [4096 k][512 b]."""
    global _last_exec_ns
    import os
    nc = _build_nc()
    bdc, bm = _make_consts()
    in_maps = []
    for c in range(NCORES):
        in_maps.append({
            "in2": _marshal_in(cols_re[c], cols_im[c]),
            "bdc": bdc, "bm": bm,
        })
    trace = bool(os.environ.get("KERNEL_TRACE"))
    try:
        res = run_bass_kernel_spmd(nc, in_maps, core_ids=list(range(NCORES)),
                                   trace=trace)
    except ModuleNotFoundError:
        # NTFF profiling hook unavailable under this axon client; run untraced.
        res = run_bass_kernel_spmd(nc, in_maps, core_ids=list(range(NCORES)))
    if trace and getattr(res, "exec_time_ns", None) is not None:
        _last_exec_ns = (_last_exec_ns or 0) + res.exec_time_ns
    return [_unmarshal_out(res.results[c]["out2"]) for c in range(NCORES)]


# ---------------------------------------------------------------------------
# public entry point
# ---------------------------------------------------------------------------

def kernel(x: np.ndarray) -> np.ndarray:
    """x: [N, 2] float32 (re, im). Returns FFT(x) as [N, 2] float32."""
    global _last_exec_ns
    _last_exec_ns = None
    x = np.asarray(x)
    Are = np.ascontiguousarray(x[:, 0].reshape(NG, NG))  # [j2g][j1g]
    Aim = np.ascontiguousarray(x[:, 1].reshape(NG, NG))

    # launch 1: FFT over rows (j2g) for each column j1g
    cols_re = [np.ascontiguousarray(Are[:, BPC * c:BPC * (c + 1)]) for c in range(NCORES)]
    cols_im = [np.ascontiguousarray(Aim[:, BPC * c:BPC * (c + 1)]) for c in range(NCORES)]
    l1 = _run_launch(cols_re, cols_im)

    # host: assemble F [k2g][j1g], twiddle, transpose-exchange
    F = np.empty((NG, NG), np.complex64)
    for c in range(NCORES):
        fre, fim = l1[c]
        F[:, BPC * c:BPC * (c + 1)] = fre + 1j * fim
    F *= _global_twiddle()

    # launch 2: FFT over j1g for each row k2g; core d gets rows [512d, 512(d+1))
    cols_re2 = []
    cols_im2 = []
    for d in range(NCORES):
        block = F[BPC * d:BPC * (d + 1), :].T      # [j1g][k2g-local]
        cols_re2.append(np.ascontiguousarray(block.real))
        cols_im2.append(np.ascontiguousarray(block.imag))
    l2 = _run_launch(cols_re2, cols_im2)

    # assemble Xmat [k1g][k2g]; out flat index k = 4096*k1g + k2g
    out = np.empty((NG, NG, 2), np.float32)
    for d in range(NCORES):
        rre, rim = l2[d]
        out[:, BPC * d:BPC * (d + 1), 0] = rre
        out[:, BPC * d:BPC * (d + 1), 1] = rim
    return out.reshape(N, 2)



# revision 36
# speedup vs baseline: 1.9911x; 1.0118x over previous
"""Distributed FFT (N = 2^24 complex points) on 8 Trainium2 NeuronCores.

Four-step (Cooley-Tukey) decomposition N = 4096 x 4096:
  launch 1: per global column j1g, FFT_4096 over j2g      (batch parallel over j1g)
  host:     global twiddle wN^{j1g*k2g} + transpose exchange
  launch 2: per global row k2g, FFT_4096 over j1g         (batch parallel over k2g)

Both launches run the SAME compiled SPMD kernel on all 8 cores: a batch of
512 local FFT_4096 per core. Each FFT_4096 = radix-32 stage (block-diag 4x
packed over the contraction axis, K=128) fused with its inter-stage transpose
(data-stationary matmul: psum[j1, :] += S_slice.T @ [[Wr|Wi],[-Wi|Wr]]),
then a radix-128 stage whose twiddle exp(-2pi i j1 kap2/4096) is folded into
32 per-kap2 weight matrices {Br, Bi, -Bi}. All arithmetic is float32r
TensorE matmuls with fp32 PSUM accumulation; all moving dims are >= 256 for
full-rate f32r.

Local FFT_4096 digits: f = j1 + 128*j2 (j1 in [0,128) fast, j2 in [0,32));
k = kap2 + 32*kap1. Batch b = 128*t + 32*g + s (t chunk of 128, g K-pack
group, s in [0,32)). Host does all layout marshalling (numpy index shuffles);
device sees only contiguous [128, X] DMAs.
"""
import numpy as np

import concourse.mybir as mybir
import concourse.tile as tile
from concourse import bacc
from concourse.bass_utils import run_bass_kernel_spmd

NG = 4096                 # global matrix dimension; N = NG*NG
N = NG * NG
NCORES = 8
BPC = NG // NCORES        # 512 signals per core per launch
import os as _os
NCHUNK = 4                # chunks of 128 signals
_ABLATE = ""              # debug ablations disabled in the shipped kernel

import ml_dtypes
_BF16_NP = ml_dtypes.bfloat16

_F32 = mybir.dt.float32
_BF16 = mybir.dt.bfloat16

# ---------------------------------------------------------------------------
# constants (host-side numpy)
# ---------------------------------------------------------------------------

_consts_cache = None


def _make_consts():
    global _consts_cache
    if _consts_cache is not None:
        return _consts_cache
    j2 = np.arange(32)
    W32 = np.exp(-2j * np.pi * np.outer(j2, j2) / 32)
    # real-matrix radix-32 DFT with re/im packed into the contraction dim:
    # rows (g, pl, j2), cols (plout, g', kap2); nonzero only for g == g'.
    # out_re = Sr@Wr - Si@Wi ; out_im = Sr@Wi + Si@Wr  -- ONE pass.
    Rsub = {(0, 0): W32.real, (1, 0): -W32.imag,
            (0, 1): W32.imag, (1, 1): W32.real}
    bdr = np.zeros((128, 128), np.float32)
    for g in range(2):
        for pl in range(2):
            for plo in range(2):
                bdr[g * 64 + pl * 32:g * 64 + pl * 32 + 32,
                    plo * 64 + g * 32:plo * 64 + g * 32 + 32] = Rsub[(pl, plo)]
    bdr = bdr.astype(_BF16_NP)

    j1 = np.arange(128)
    W128 = np.exp(-2j * np.pi * np.outer(j1, j1) / 128)
    bm = np.zeros((32, 2, 128, 128), np.float32)
    for kap2 in range(32):
        B = np.exp(-2j * np.pi * j1 * kap2 / 4096)[:, None] * W128  # [j1][kap1]
        bm[kap2, 0] = B.real
        bm[kap2, 1] = B.imag
    # p-major DRAM layout [p=128, kap2=32, v=2, kap1=128] so the device DMA
    # is one contiguous run per partition (16 KiB in bf16).
    bm = np.ascontiguousarray(bm.transpose(2, 0, 1, 3)).astype(_BF16_NP)
    _consts_cache = (bdr, bm)
    return _consts_cache


_tw_cache = None


def _global_twiddle():
    """exp(-2pi i k2g*j1g / N) as complex64 [NG, NG] (k2g rows)."""
    global _tw_cache
    if _tw_cache is None:
        k = np.arange(NG, dtype=np.float64)
        phase = np.outer(k, k) * (-2.0 * np.pi / N)
        _tw_cache = np.exp(1j * phase).astype(np.complex64)
    return _tw_cache


# ---------------------------------------------------------------------------
# marshalling (host)
# ---------------------------------------------------------------------------

def _marshal_in(Vre, Vim):
    """Vre/Vim: [4096 f][512 b] f32 planes -> in2 [8,128,4096] bf16.
    chunk tg = 2t+h ; partitions = 64*gp + 32*pl + j2 ; free = 128*s + j1 ;
    signal b = 128*t + 32*(2h+gp) + s."""
    out = np.empty((4, 2, 2, 2, 32, 32, 128), _BF16_NP)  # t,h,gp,pl,j2,s,j1
    ov = out
    for pl, V in ((0, Vre), (1, Vim)):
        V2 = V.reshape(32, 128, 4, 2, 2, 32)   # j2, j1, t, h, gp, s
        ov[:, :, :, pl] = V2.transpose(2, 3, 4, 0, 5, 1).astype(_BF16_NP)
    return out.reshape(8, 128, 4096)


def _unmarshal_out(O):
    """out2 [2,16,2,128,512] bf16 (dims sc,kp,pl,kap1,n2; n2=256u+128c2+4s+g)
    -> (Fre, Fim) planes [4096 k][512 b]."""
    O = np.asarray(O).astype(np.float32)
    O8 = O.reshape(2, 16, 2, 128, 2, 2, 32, 4)  # sc, kp, pl, kap1, u, c2, s, g
    # kap2 = 2*kp+u ; k = 32*kap1 + kap2 ; b = 256*sc + 128*c2 + 32*g + s
    P = np.ascontiguousarray(O8.transpose(2, 3, 1, 4, 0, 5, 7, 6)).reshape(2, 4096, 512)
    return P[0], P[1]


# ---------------------------------------------------------------------------
# device kernel (Bass/Tile), shared by both launches
# ---------------------------------------------------------------------------

_nc_cache = None


def _build_nc():
    global _nc_cache
    if _nc_cache is not None:
        return _nc_cache

    nc = bacc.Bacc(trn_type="TRN2")
    DT = _BF16
    # in layout: [chunk tg, p = 64gp+32pl+j2, ff = 128s+j1]
    in_d = nc.dram_tensor("in2", [8, 128, 4096], DT, kind="ExternalInput")
    bdr_d = nc.dram_tensor("bdr", [128, 128], DT, kind="ExternalInput")
    bm_d = nc.dram_tensor("bm", [128, 32, 2, 128], DT, kind="ExternalInput")
    # out layout: [superchunk, kap2pair, plane, kap1, n2], n2 = 128*c2 + 4*s + g
    out_d = nc.dram_tensor("out2", [NCHUNK // 2, 16, 2, 128, 512], DT,
                           kind="ExternalOutput")

    with tile.TileContext(nc) as tc:
        with (
            tc.tile_pool(name="consts", bufs=1) as cpool,
            tc.tile_pool(name="inp", bufs=4) as inpool,
            tc.tile_pool(name="tp", bufs=2) as tpool,
            tc.tile_pool(name="outp", bufs=8) as outpool,
            tc.tile_pool(name="pp", bufs=8, space="PSUM") as pp,
        ):
            st = {}

            def load_chunk(t, first=False):
                if t in st or t >= 8:
                    return
                s = inpool.tile([128, 4096], DT, tag="in")
                # piece-DMAs so stage A can start on the first piece early;
                # the (tiny) bdr const rides the Pool/SWDGE path in parallel
                if first:
                    nc.gpsimd.dma_start(bdr_t[:], bdr_d[:])
                    bounds = [0, 384, 768, 1152, 1664, 2176, 2688, 3200, 3712, 4096]
                    for i in range(len(bounds) - 1):
                        qs = slice(bounds[i], bounds[i + 1])
                        nc.sync.dma_start(s[:, qs], in_d[t, :, qs])
                else:
                    for q in range(2):
                        qs = slice(2048 * q, 2048 * q + 2048)
                        nc.sync.dma_start(s[:, qs], in_d[t, :, qs])
                st[t] = s

            bdr_t = cpool.tile([128, 128], DT, tag="bdr")
            load_chunk(0, first=True)
            load_chunk(1)
            load_chunk(2)
            load_chunk(3)

            # bm streams in 4 kap2-groups interleaved with chunk prefetches,
            # each arriving just before stage B consumes it. Only {br, bi}
            # come over DMA; the -bi slab is synthesized in SBUF by one DVE
            # negate per group (PSUM-operand rules forbid p1-p2 evacuation).
            bm_t = cpool.tile([128, 32, 2, 128], DT, tag="bm")
            bnm_t = cpool.tile([128, 32, 128], DT, tag="bnm")
            bm_loaded = [False] * 4

            def load_bm_group(g):
                if g >= 4 or bm_loaded[g]:
                    return
                ks = slice(8 * g, 8 * g + 8)
                nc.sync.dma_start(bm_t[:, ks], bm_d[:, ks])
                nc.vector.tensor_scalar_mul(bnm_t[:, ks], bm_t[:, ks, 1], -1.0)
                bm_loaded[g] = True

            load_bm_group(0)

            ncopy = 0  # alternate DVE/ACT for all PSUM evacuations

            def evac(out_ap, in_ap):
                nonlocal ncopy
                if ncopy % 2 == 0:
                    nc.vector.tensor_copy(out_ap, in_ap)
                else:
                    nc.scalar.copy(out_ap, in_ap)
                ncopy += 1

            def make_tt():
                # T for a superchunk: [p=j1][c2][s][plane][g][kap]
                tt = tpool.tile([128, 2, 32, 2, 4, 32], DT, tag="tt")
                ttf = tt.rearrange("p a b c d e -> p (a b c d e)")
                return tt, ttf

            def emit_A_bank(tt, tg, s_tile, q):
                # one-pass real-matrix radix-32: psum[j1, (plout, gp, kap2)]
                # = S[(gp,pl,j2), j1].T @ bdr ; four s per PSUM bank
                c2 = (tg // 2) % 2
                h = tg % 2
                bank = pp.tile([128, 512], _F32, tag="ps")
                for i in range(4):
                    sl = 4 * q + i
                    ds = slice(128 * sl, 128 * sl + 128)
                    nc.tensor.matmul(bank[:, 128 * i:128 * i + 128],
                                     s_tile[:, ds], bdr_t[:],
                                     start=True, stop=True)
                dst = tt[:, c2, 4 * q:4 * q + 4, :, 2 * h:2 * h + 2, :]
                evac(dst, bank[:])

            def emit_B_kp(tt, sc, kp):
                # stage B: radix-128, per-kap2 twiddled weights, N=256
                ot = outpool.tile([128, 2, 512], DT, tag="out")
                yr = pp.tile([128, 512], _F32, tag="ps")
                yi = pp.tile([128, 512], _F32, tag="ps")
                for u in range(2):
                    kap2 = 2 * kp + u
                    ys = slice(256 * u, 256 * u + 256)
                    trs = tt[:, :, :, 0, :, kap2]
                    tis = tt[:, :, :, 1, :, kap2]
                    br = bm_t[:, kap2, 0]
                    bi = bm_t[:, kap2, 1]
                    bni = bnm_t[:, kap2]
                    nc.tensor.matmul(yr[:, ys], br, trs, start=True, stop=False)
                    nc.tensor.matmul(yi[:, ys], br, tis, start=True, stop=False)
                    nc.tensor.matmul(yr[:, ys], bni, tis, start=False, stop=True)
                    nc.tensor.matmul(yi[:, ys], bi, trs, start=False, stop=True)
                evac(ot[:, 0], yr[:])
                evac(ot[:, 1], yi[:])
                nc.sync.dma_start(
                    out_d[sc, kp].rearrange("a p f -> p a f"), ot[:])

            # Emission: A(sc0) over chunks 0-3, then B(sc0) interleaved 2
            # A-banks : 1 B-kp with A(sc1) over chunks 4-7, then B(sc1).
            tt0, ttf0 = make_tt()
            for tg in range(4):
                s_t = st.pop(tg)
                for q in range(8):
                    emit_A_bank(tt0, tg, s_t, q)

            tt1, ttf1 = make_tt()
            load_chunk(4)
            load_bm_group(1)
            load_bm_group(2)
            load_bm_group(3)
            load_chunk(5)
            load_chunk(6)
            load_chunk(7)
            sc1_units = [(tg, q) for tg in range(4, 8) for q in range(8)]
            for kp in range(16):
                for j in range(2):
                    tg, q = sc1_units[2 * kp + j]
                    if q == 0:
                        st_cur = st.pop(tg)
                        st[tg] = st_cur
                    emit_A_bank(tt1, tg, st[tg], q)
                emit_B_kp(tt0, 0, kp)
            for tg in range(4, 8):
                st.pop(tg, None)
            for kp in range(16):
                emit_B_kp(tt1, 1, kp)


    nc.finalize()
    _nc_cache = nc
    return nc


# ---------------------------------------------------------------------------
# launch helper
# ---------------------------------------------------------------------------

_last_exec_ns = None


def last_exec_time_ns():
    """Sum of HW exec times (ns) of the launches in the last kernel() call,
    when KERNEL_TRACE=1 was set and NTFF profiling is available. None otherwise."""
    return _last_exec_ns


def predicted_exec_time_ns():
    """Cost-model (TimelineSim) predicted HW exec time for both launches, ns."""
    from concourse.timeline_sim import TimelineSim
    nc = _build_nc()
    return int(2 * TimelineSim(nc).simulate())


def _run_launch(cols_re, cols_im):
    """cols_re/cols_im: list of 8 planes [4096 f][512 b] f32.
    Returns list of 8 (Fre, Fim) planes [4096 k][512 b]."""
    global _last_exec_ns
    import os
    nc = _build_nc()
    bdr, bm = _make_consts()
    in_maps = []
    for c in range(NCORES):
        in_maps.append({
            "in2": _marshal_in(cols_re[c], cols_im[c]),
            "bdr": bdr, "bm": bm,
        })
    trace = bool(os.environ.get("KERNEL_TRACE"))
    try:
        res = run_bass_kernel_spmd(nc, in_maps, core_ids=list(range(NCORES)),
                                   trace=trace)
    except ModuleNotFoundError:
        # NTFF profiling hook unavailable under this axon client; run untraced.
        res = run_bass_kernel_spmd(nc, in_maps, core_ids=list(range(NCORES)))
    if trace and getattr(res, "exec_time_ns", None) is not None:
        _last_exec_ns = (_last_exec_ns or 0) + res.exec_time_ns
    return [_unmarshal_out(res.results[c]["out2"]) for c in range(NCORES)]


# ---------------------------------------------------------------------------
# public entry point
# ---------------------------------------------------------------------------

def kernel(x: np.ndarray) -> np.ndarray:
    """x: [N, 2] float32 (re, im). Returns FFT(x) as [N, 2] float32."""
    global _last_exec_ns
    _last_exec_ns = None
    x = np.asarray(x)
    Are = np.ascontiguousarray(x[:, 0].reshape(NG, NG))  # [j2g][j1g]
    Aim = np.ascontiguousarray(x[:, 1].reshape(NG, NG))

    # launch 1: FFT over rows (j2g) for each column j1g
    cols_re = [np.ascontiguousarray(Are[:, BPC * c:BPC * (c + 1)]) for c in range(NCORES)]
    cols_im = [np.ascontiguousarray(Aim[:, BPC * c:BPC * (c + 1)]) for c in range(NCORES)]
    l1 = _run_launch(cols_re, cols_im)

    # host: assemble F [k2g][j1g], twiddle, transpose-exchange
    F = np.empty((NG, NG), np.complex64)
    for c in range(NCORES):
        fre, fim = l1[c]
        F[:, BPC * c:BPC * (c + 1)] = fre + 1j * fim
    F *= _global_twiddle()

    # launch 2: FFT over j1g for each row k2g; core d gets rows [512d, 512(d+1))
    cols_re2 = []
    cols_im2 = []
    for d in range(NCORES):
        block = F[BPC * d:BPC * (d + 1), :].T      # [j1g][k2g-local]
        cols_re2.append(np.ascontiguousarray(block.real))
        cols_im2.append(np.ascontiguousarray(block.imag))
    l2 = _run_launch(cols_re2, cols_im2)

    # assemble Xmat [k1g][k2g]; out flat index k = 4096*k1g + k2g
    out = np.empty((NG, NG, 2), np.float32)
    for d in range(NCORES):
        rre, rim = l2[d]
        out[:, BPC * d:BPC * (d + 1), 0] = rre
        out[:, BPC * d:BPC * (d + 1), 1] = rim
    return out.reshape(N, 2)



# revision 37
# speedup vs baseline: 2.2104x; 1.1101x over previous
"""Distributed FFT (N = 2^24 complex points) on 8 Trainium2 NeuronCores.

Four-step (Cooley-Tukey) decomposition N = 4096 x 4096:
  launch 1: per global column j1g, FFT_4096 over j2g      (batch parallel over j1g)
  host:     global twiddle wN^{j1g*k2g} + transpose exchange
  launch 2: per global row k2g, FFT_4096 over j1g         (batch parallel over k2g)

Both launches run the SAME compiled SPMD kernel on all 8 cores: a batch of
512 local FFT_4096 per core. Each FFT_4096 = radix-32 stage (block-diag 4x
packed over the contraction axis, K=128) fused with its inter-stage transpose
(data-stationary matmul: psum[j1, :] += S_slice.T @ [[Wr|Wi],[-Wi|Wr]]),
then a radix-128 stage whose twiddle exp(-2pi i j1 kap2/4096) is folded into
32 per-kap2 weight matrices {Br, Bi, -Bi}. All arithmetic is float32r
TensorE matmuls with fp32 PSUM accumulation; all moving dims are >= 256 for
full-rate f32r.

Local FFT_4096 digits: f = j1 + 128*j2 (j1 in [0,128) fast, j2 in [0,32));
k = kap2 + 32*kap1. Batch b = 128*t + 32*g + s (t chunk of 128, g K-pack
group, s in [0,32)). Host does all layout marshalling (numpy index shuffles);
device sees only contiguous [128, X] DMAs.
"""
import numpy as np

import concourse.mybir as mybir
import concourse.tile as tile
from concourse import bacc
from concourse.bass_utils import run_bass_kernel_spmd

NG = 4096                 # global matrix dimension; N = NG*NG
N = NG * NG
NCORES = 8
BPC = NG // NCORES        # 512 signals per core per launch
import os as _os
NCHUNK = 4                # chunks of 128 signals
_ABLATE = ""              # debug ablations disabled in the shipped kernel

import ml_dtypes
_BF16_NP = ml_dtypes.bfloat16

_F32 = mybir.dt.float32
_BF16 = mybir.dt.bfloat16

# ---------------------------------------------------------------------------
# constants (host-side numpy)
# ---------------------------------------------------------------------------

_consts_cache = None


def _make_consts():
    global _consts_cache
    if _consts_cache is not None:
        return _consts_cache
    j2 = np.arange(32)
    W32 = np.exp(-2j * np.pi * np.outer(j2, j2) / 32)
    # real-matrix radix-32 DFT with re/im packed into the contraction dim:
    # rows (g, pl, j2), cols (plout, g', kap2); nonzero only for g == g'.
    # out_re = Sr@Wr - Si@Wi ; out_im = Sr@Wi + Si@Wr  -- ONE pass.
    Rsub = {(0, 0): W32.real, (1, 0): -W32.imag,
            (0, 1): W32.imag, (1, 1): W32.real}
    bdr = np.zeros((128, 128), np.float32)
    for g in range(2):
        for pl in range(2):
            for plo in range(2):
                bdr[g * 64 + pl * 32:g * 64 + pl * 32 + 32,
                    plo * 64 + g * 32:plo * 64 + g * 32 + 32] = Rsub[(pl, plo)]
    bdr = bdr.astype(_BF16_NP)

    j1 = np.arange(128)
    W128 = np.exp(-2j * np.pi * np.outer(j1, j1) / 128)
    bm = np.zeros((32, 2, 128, 128), np.float32)
    for kap2 in range(32):
        B = np.exp(-2j * np.pi * j1 * kap2 / 4096)[:, None] * W128  # [j1][kap1]
        bm[kap2, 0] = B.real
        bm[kap2, 1] = B.imag
    # p-major DRAM layout [p=128, kap2=32, v=2, kap1=128] so the device DMA
    # is one contiguous run per partition (16 KiB in bf16).
    bm = np.ascontiguousarray(bm.transpose(2, 0, 1, 3)).astype(_BF16_NP)
    _consts_cache = (bdr, bm)
    return _consts_cache


_tw_cache = None


def _global_twiddle():
    """exp(-2pi i k2g*j1g / N) as complex64 [NG, NG] (k2g rows)."""
    global _tw_cache
    if _tw_cache is None:
        k = np.arange(NG, dtype=np.float64)
        phase = np.outer(k, k) * (-2.0 * np.pi / N)
        _tw_cache = np.exp(1j * phase).astype(np.complex64)
    return _tw_cache


# ---------------------------------------------------------------------------
# marshalling (host)
# ---------------------------------------------------------------------------

def _marshal_in(Vre, Vim):
    """Vre/Vim: [4096 f][512 b] f32 planes -> in2 [8,128,4096] bf16.
    chunk tg = 2t+h ; partitions = 64*gp + 32*pl + j2 ; free = 128*s + j1 ;
    signal b = 128*t + 32*(2h+gp) + s."""
    out = np.empty((4, 2, 2, 2, 32, 32, 128), _BF16_NP)  # t,h,gp,pl,j2,s,j1
    ov = out
    for pl, V in ((0, Vre), (1, Vim)):
        V2 = V.reshape(32, 128, 4, 2, 2, 32)   # j2, j1, t, h, gp, s
        ov[:, :, :, pl] = V2.transpose(2, 3, 4, 0, 5, 1).astype(_BF16_NP)
    return out.reshape(8, 128, 4096)


def _unmarshal_out(O):
    """out2 [2,16,2,128,512] bf16 (dims sc,kp,pl,kap1,n2; n2=256u+128c2+4s+g)
    -> (Fre, Fim) planes [4096 k][512 b]."""
    O = np.asarray(O).astype(np.float32)
    O8 = O.reshape(2, 16, 2, 128, 2, 2, 32, 4)  # sc, kp, pl, kap1, u, c2, s, g
    # kap2 = 2*kp+u ; k = 32*kap1 + kap2 ; b = 256*sc + 128*c2 + 32*g + s
    P = np.ascontiguousarray(O8.transpose(2, 3, 1, 4, 0, 5, 7, 6)).reshape(2, 4096, 512)
    return P[0], P[1]


# ---------------------------------------------------------------------------
# device kernel (Bass/Tile), shared by both launches
# ---------------------------------------------------------------------------

_nc_cache = None


def _build_nc():
    global _nc_cache
    if _nc_cache is not None:
        return _nc_cache

    nc = bacc.Bacc(trn_type="TRN2")
    DT = _BF16
    # in layout: [chunk tg, p = 64gp+32pl+j2, ff = 128s+j1]
    in_d = nc.dram_tensor("in2", [8, 128, 4096], DT, kind="ExternalInput")
    bdr_d = nc.dram_tensor("bdr", [128, 128], DT, kind="ExternalInput")
    bm_d = nc.dram_tensor("bm", [128, 32, 2, 128], DT, kind="ExternalInput")
    # out layout: [superchunk, kap2pair, plane, kap1, n2], n2 = 128*c2 + 4*s + g
    out_d = nc.dram_tensor("out2", [NCHUNK // 2, 16, 2, 128, 512], DT,
                           kind="ExternalOutput")

    with tile.TileContext(nc) as tc:
        with (
            tc.tile_pool(name="consts", bufs=1) as cpool,
            tc.tile_pool(name="inp", bufs=4) as inpool,
            tc.tile_pool(name="tp", bufs=2) as tpool,
            tc.tile_pool(name="outp", bufs=16) as outpool,
            tc.tile_pool(name="pp", bufs=8, space="PSUM") as pp,
        ):
            st = {}

            def load_chunk(t, first=False):
                if t in st or t >= 8:
                    return
                s = inpool.tile([128, 4096], DT, tag="in")
                # DMA-bound regime: big pieces keep the DMA stream dense;
                # the (tiny) bdr const rides the Pool/SWDGE path in parallel
                if first:
                    nc.gpsimd.dma_start(bdr_t[:], bdr_d[:])
                for q in range(2):
                    qs = slice(2048 * q, 2048 * q + 2048)
                    nc.sync.dma_start(s[:, qs], in_d[t, :, qs])
                st[t] = s

            bdr_t = cpool.tile([128, 128], DT, tag="bdr")
            load_chunk(0, first=True)
            load_chunk(1)
            load_chunk(2)
            load_chunk(3)

            # bm streams in 4 kap2-groups interleaved with chunk prefetches,
            # each arriving just before stage B consumes it. Only {br, bi}
            # come over DMA; the -bi slab is synthesized in SBUF by one DVE
            # negate per group (PSUM-operand rules forbid p1-p2 evacuation).
            bm_t = cpool.tile([128, 32, 2, 128], DT, tag="bm")
            bnm_t = cpool.tile([128, 32, 128], DT, tag="bnm")
            bm_loaded = [False] * 4

            def load_bm_group(g):
                if g >= 4 or bm_loaded[g]:
                    return
                ks = slice(8 * g, 8 * g + 8)
                nc.sync.dma_start(bm_t[:, ks], bm_d[:, ks])
                nc.vector.tensor_scalar_mul(bnm_t[:, ks], bm_t[:, ks, 1], -1.0)
                bm_loaded[g] = True

            load_bm_group(0)

            ncopy = 0  # alternate DVE/ACT for all PSUM evacuations

            def evac(out_ap, in_ap):
                nonlocal ncopy
                if ncopy % 2 == 0:
                    nc.vector.tensor_copy(out_ap, in_ap)
                else:
                    nc.scalar.copy(out_ap, in_ap)
                ncopy += 1

            def make_tt():
                # T for a superchunk: [p=j1][c2][s][plane][g][kap]
                tt = tpool.tile([128, 2, 32, 2, 4, 32], DT, tag="tt")
                ttf = tt.rearrange("p a b c d e -> p (a b c d e)")
                return tt, ttf

            def emit_A_bank(tt, tg, s_tile, q):
                # one-pass real-matrix radix-32: psum[j1, (plout, gp, kap2)]
                # = S[(gp,pl,j2), j1].T @ bdr ; four s per PSUM bank
                c2 = (tg // 2) % 2
                h = tg % 2
                bank = pp.tile([128, 512], _F32, tag="ps")
                for i in range(4):
                    sl = 4 * q + i
                    ds = slice(128 * sl, 128 * sl + 128)
                    nc.tensor.matmul(bank[:, 128 * i:128 * i + 128],
                                     s_tile[:, ds], bdr_t[:],
                                     start=True, stop=True)
                dst = tt[:, c2, 4 * q:4 * q + 4, :, 2 * h:2 * h + 2, :]
                evac(dst, bank[:])

            def emit_B_kp(tt, sc, kp):
                # stage B: radix-128, per-kap2 twiddled weights, N=256
                ot = outpool.tile([128, 2, 512], DT, tag="out")
                yr = pp.tile([128, 512], _F32, tag="ps")
                yi = pp.tile([128, 512], _F32, tag="ps")
                for u in range(2):
                    kap2 = 2 * kp + u
                    ys = slice(256 * u, 256 * u + 256)
                    trs = tt[:, :, :, 0, :, kap2]
                    tis = tt[:, :, :, 1, :, kap2]
                    br = bm_t[:, kap2, 0]
                    bi = bm_t[:, kap2, 1]
                    bni = bnm_t[:, kap2]
                    nc.tensor.matmul(yr[:, ys], br, trs, start=True, stop=False)
                    nc.tensor.matmul(yi[:, ys], br, tis, start=True, stop=False)
                    nc.tensor.matmul(yr[:, ys], bni, tis, start=False, stop=True)
                    nc.tensor.matmul(yi[:, ys], bi, trs, start=False, stop=True)
                evac(ot[:, 0], yr[:])
                evac(ot[:, 1], yi[:])
                nc.sync.dma_start(
                    out_d[sc, kp].rearrange("a p f -> p a f"), ot[:])

            # Emission: A(sc0) over chunks 0-3, then B(sc0) interleaved 2
            # A-banks : 1 B-kp with A(sc1) over chunks 4-7, then B(sc1).
            tt0, ttf0 = make_tt()
            for tg in range(4):
                s_t = st.pop(tg)
                for q in range(8):
                    emit_A_bank(tt0, tg, s_t, q)

            tt1, ttf1 = make_tt()
            load_bm_group(1)
            load_chunk(4)
            load_chunk(5)
            load_bm_group(2)
            load_chunk(6)
            load_bm_group(3)
            load_chunk(7)
            # front-load B0 kps (they only need tt0 + bm), then pace A1
            # banks to chunk arrival with the remaining B0 kps as filler
            for kp in range(5):
                emit_B_kp(tt0, 0, kp)
            sc1_units = [(tg, q) for tg in range(4, 8) for q in range(8)]
            for i in range(8):
                for j in range(4):
                    tg, q = sc1_units[4 * i + j]
                    emit_A_bank(tt1, tg, st[tg], q)
                if 5 + i < 16:
                    emit_B_kp(tt0, 0, 5 + i)
            for kp in range(13, 16):
                emit_B_kp(tt0, 0, kp)
            for tg in range(4, 8):
                st.pop(tg, None)
            for kp in range(16):
                emit_B_kp(tt1, 1, kp)


    nc.finalize()
    _nc_cache = nc
    return nc


# ---------------------------------------------------------------------------
# launch helper
# ---------------------------------------------------------------------------

_last_exec_ns = None


def last_exec_time_ns():
    """Sum of HW exec times (ns) of the launches in the last kernel() call,
    when KERNEL_TRACE=1 was set and NTFF profiling is available. None otherwise."""
    return _last_exec_ns


def predicted_exec_time_ns():
    """Cost-model (TimelineSim) predicted HW exec time for both launches, ns."""
    from concourse.timeline_sim import TimelineSim
    nc = _build_nc()
    return int(2 * TimelineSim(nc).simulate())


def _run_launch(cols_re, cols_im):
    """cols_re/cols_im: list of 8 planes [4096 f][512 b] f32.
    Returns list of 8 (Fre, Fim) planes [4096 k][512 b]."""
    global _last_exec_ns
    import os
    nc = _build_nc()
    bdr, bm = _make_consts()
    in_maps = []
    for c in range(NCORES):
        in_maps.append({
            "in2": _marshal_in(cols_re[c], cols_im[c]),
            "bdr": bdr, "bm": bm,
        })
    trace = bool(os.environ.get("KERNEL_TRACE"))
    try:
        res = run_bass_kernel_spmd(nc, in_maps, core_ids=list(range(NCORES)),
                                   trace=trace)
    except ModuleNotFoundError:
        # NTFF profiling hook unavailable under this axon client; run untraced.
        res = run_bass_kernel_spmd(nc, in_maps, core_ids=list(range(NCORES)))
    if trace and getattr(res, "exec_time_ns", None) is not None:
        _last_exec_ns = (_last_exec_ns or 0) + res.exec_time_ns
    return [_unmarshal_out(res.results[c]["out2"]) for c in range(NCORES)]


# ---------------------------------------------------------------------------
# public entry point
# ---------------------------------------------------------------------------

def kernel(x: np.ndarray) -> np.ndarray:
    """x: [N, 2] float32 (re, im). Returns FFT(x) as [N, 2] float32."""
    global _last_exec_ns
    _last_exec_ns = None
    x = np.asarray(x)
    Are = np.ascontiguousarray(x[:, 0].reshape(NG, NG))  # [j2g][j1g]
    Aim = np.ascontiguousarray(x[:, 1].reshape(NG, NG))

    # launch 1: FFT over rows (j2g) for each column j1g
    cols_re = [np.ascontiguousarray(Are[:, BPC * c:BPC * (c + 1)]) for c in range(NCORES)]
    cols_im = [np.ascontiguousarray(Aim[:, BPC * c:BPC * (c + 1)]) for c in range(NCORES)]
    l1 = _run_launch(cols_re, cols_im)

    # host: assemble F [k2g][j1g], twiddle, transpose-exchange
    F = np.empty((NG, NG), np.complex64)
    for c in range(NCORES):
        fre, fim = l1[c]
        F[:, BPC * c:BPC * (c + 1)] = fre + 1j * fim
    F *= _global_twiddle()

    # launch 2: FFT over j1g for each row k2g; core d gets rows [512d, 512(d+1))
    cols_re2 = []
    cols_im2 = []
    for d in range(NCORES):
        block = F[BPC * d:BPC * (d + 1), :].T      # [j1g][k2g-local]
        cols_re2.append(np.ascontiguousarray(block.real))
        cols_im2.append(np.ascontiguousarray(block.imag))
    l2 = _run_launch(cols_re2, cols_im2)

    # assemble Xmat [k1g][k2g]; out flat index k = 4096*k1g + k2g
    out = np.empty((NG, NG, 2), np.float32)
    for d in range(NCORES):
        rre, rim = l2[d]
        out[:, BPC * d:BPC * (d + 1), 0] = rre
        out[:, BPC * d:BPC * (d + 1), 1] = rim
    return out.reshape(N, 2)

